# revision 1
# baseline (speedup 1.0000x reference)
"""TRN2 Bass kernel for nn_Block_82325933129820.

3x AFT blocks + 1 transformer (TEA) block, B=4 T=1024 E=1024 QKV=2048 H=16.

Sharding: 8 cores = 4 batch-pairs. Within a pair (even core, odd core):
  - AFT layers: token-split (even: tokens 0-511, odd: 512-1023), feature-major
    activations (channels on partitions, tokens on free dim). The cumsum runs
    as per-chunk tensor_tensor_scan along the free dim; cross-core carries
    (the even core's token-sums, from activation/STT accum_out) travel via
    pair AllGathers (4 groups/layer for pipelining) and enter as the scan's
    `initial` value, gated to zero on even cores.
  - TEA: head-split (even: heads 0-7, odd: 8-15) over the full 1024 tokens.
    x3 is pair-AllGathered; attention is computed in S^T layout (k on
    partitions, q on free) so softmax sums use ones-matmul broadcasts; the
    swiglu is computed as a partial contraction over each core's own y
    channels for ALL tokens, then a pair ReduceScatter simultaneously sums
    the partials and re-shards by token half.

All matmuls run in float32r (full PE rate at N>=256). Only ACT table used is
natural_log_exp_and_others: exp directly; sigmoid/silu via exp with the
reciprocal folded into existing divisions; rsqrt(x) = Exp(-0.5*Ln(x)).
"""
import os
import sys
import numpy as np

for _p in ('/opt/trn_rl_repo',):
    if _p not in sys.path:
        sys.path.insert(0, _p)

import concourse.bass as bass
import concourse.mybir as mybir
import concourse.tile as tile
from concourse import bacc
from concourse.bass_utils import run_bass_kernel_spmd

P = 128
TL = 512          # AFT tokens per core
E = 1024
QKV = 2048
T = 1024
DH = 128
NCORES = 8
NE = E // P       # 8
NC = QKV // P     # 16
EPS = float(np.finfo(np.float32).eps)
f32 = mybir.dt.float32
f32r = mybir.dt.float32r
AF = mybir.ActivationFunctionType
ALU = mybir.AluOpType
PAIRS = [[0, 1], [2, 3], [4, 5], [6, 7]]


def _rsqrt(nc, pool, src_ps, scale, bias_ap, tag, ln_bufs=None):
    """rsqrt(src*scale + bias) = Exp(-0.5*Ln(.)). src_ps is PSUM (P, TL)."""
    tmp = pool.tile([P, TL], f32, tag="lntmp", bufs=ln_bufs)
    nc.scalar.activation(tmp[:], src_ps[:], AF.Ln, scale=scale, bias=bias_ap)
    out = pool.tile([P, TL], f32, tag=tag)
    nc.scalar.activation(out[:], tmp[:], AF.Exp, scale=-0.5)
    return out


def _wtile8(nc, pool, wdram, m):
    """(P, 8, P) f32r weight tile m from host-pretiled (M_total, K=8P) DRAM:
    row-block m is the contiguous [p, e, n] tile."""
    wt = pool.tile([P, NE, P], f32r, tag="wk8")
    nc.sync.dma_start(wt[:], wdram.ap()[m * P:(m + 1) * P, :]
                      .rearrange("p (a n) -> p a n", n=P))
    return wt


def _build_aft_layer(tc, const, x_ap, wqkvT, wswiT, woutT,
                     ag_ins, ag_outs, q_dram, xout):
    """One AFT layer. x_ap: (E, TL) f32 residual AP in DRAM. Writes xout AP."""
    nc = tc.nc
    ones_r = const["ones_r"]
    gate_col = const["gate"]

    with (
        tc.tile_pool(name="a_yf", bufs=NC) as yfp,
        tc.tile_pool(name="a_sc", bufs=2) as scp,
        tc.tile_pool(name="a_cc", bufs=20) as ccp,
        tc.tile_pool(name="a_ps", bufs=3, space="PSUM") as ps,
        tc.tile_pool(name="a_ps2", bufs=1, space="PSUM") as ps2,
    ):
        yf_t = [None] * NC
        with (
            tc.tile_pool(name="a_k", bufs=NC) as kp,
            tc.tile_pool(name="a_wwv", bufs=10) as wwvp,
            tc.tile_pool(name="a_pb", bufs=2) as pbp,
        ):
            with (
                tc.tile_pool(name="a_xn", bufs=NE) as xnp,
                tc.tile_pool(name="a_w8", bufs=3) as wp,
                tc.tile_pool(name="a_ld", bufs=2) as sbp,
            ):
                # ---- rms(x) ----
                sumsq = ps2.tile([P, TL], f32, tag="xsumsq")
                for e in range(NE):
                    xl = sbp.tile([P, TL], f32, tag="xl")
                    nc.sync.dma_start(xl[:], x_ap[e * P:(e + 1) * P, :])
                    xsq = sbp.tile([P, TL], f32r, tag="sq")
                    nc.scalar.activation(xsq[:], xl[:], AF.Square)
                    nc.tensor.matmul(sumsq[:], ones_r[:], xsq[:],
                                     start=(e == 0), stop=(e == NE - 1))
                xscale = _rsqrt(nc, scp, sumsq, 1.0 / E, const["epsb"][:],
                                "scale")
                xn = []
                for e in range(NE):
                    xl = sbp.tile([P, TL], f32, tag="xl")
                    nc.sync.dma_start(xl[:], x_ap[e * P:(e + 1) * P, :])
                    t = xnp.tile([P, TL], f32r, tag="xn")
                    nc.vector.tensor_tensor(t[:], xl[:], xscale[:],
                                            ALU.mult)
                    xn.append(t)

                def qkv_mtile(m, tag="mm"):
                    wt = _wtile8(nc, wp, wqkvT, m)
                    acc = ps.tile([P, TL], f32, tag=tag,
                                  bufs=(2 if tag == "mmq" else None))
                    for e in range(NE):
                        nc.tensor.matmul(acc[:], wt[:, e, :], xn[e][:],
                                         start=(e == 0), stop=(e == NE - 1))
                    return acc

                # ---- k tiles (SBUF-resident) ----
                k_sb = [None] * NC
                ksumsq = ps2.tile([P, TL], f32, tag="ksumsq")
                for c in range(NC):
                    acc = qkv_mtile(16 + c)
                    kt = kp.tile([P, TL], f32, tag="k")
                    nc.scalar.copy(kt[:], acc[:])
                    k_sb[c] = kt
                    ksq = sbp.tile([P, TL], f32r, tag="sq")
                    nc.scalar.activation(ksq[:], acc[:], AF.Square)
                    nc.tensor.matmul(ksumsq[:], ones_r[:], ksq[:],
                                     start=(c == 0), stop=(c == NC - 1))
                kscale = _rsqrt(nc, scp, ksumsq, 1.0 / QKV, const["epsb"][:],
                                "scale")

                # ---- v + w/wv + carries (2 groups of 8), then q ----
                w_t = [None] * NC
                wv_t = [None] * NC
                for g in range(2):
                    for c in range(8 * g, 8 * g + 8):
                        acc = qkv_mtile(32 + c)
                        kn = sbp.tile([P, TL], f32, tag="kn")
                        nc.vector.tensor_tensor(kn[:], k_sb[c][:], kscale[:],
                                                ALU.mult)
                        w = wwvp.tile([P, TL], f32, tag="w")
                        cw_col = ccp.tile([P, 1], f32, tag="cwc")
                        nc.scalar.activation(w[:], kn[:], AF.Exp,
                                             accum_out=cw_col[:])
                        wv = wwvp.tile([P, TL], f32, tag="wv")
                        cwv_col = ccp.tile([P, 1], f32, tag="cwvc")
                        nc.vector.scalar_tensor_tensor(
                            wv[:], acc[:], 0.0, w[:], ALU.bypass, ALU.mult,
                            accum_out=cwv_col[:])
                        j = c - 8 * g
                        nc.sync.dma_start(
                            ag_ins[g].opt()[:, j * P:(j + 1) * P]
                            .rearrange("o (p q) -> p (o q)", p=P),
                            cwv_col[:])
                        nc.sync.dma_start(
                            ag_ins[g].opt()[:, 1024 + j * P:1024 + (j + 1) * P]
                            .rearrange("o (p q) -> p (o q)", p=P),
                            cw_col[:])
                        w_t[c] = w
                        wv_t[c] = wv
                    nc.gpsimd.collective_compute(
                        "AllGather", ALU.bypass, replica_groups=PAIRS,
                        ins=[ag_ins[g].opt()], outs=[ag_outs[g].opt()])

                qsumsq = ps2.tile([P, TL], f32, tag="qsumsq")
                for c in range(NC):
                    acc = qkv_mtile(c, tag="mmq")
                    qt = sbp.tile([P, TL], f32, tag="xl")
                    nc.scalar.copy(qt[:], acc[:])
                    nc.sync.dma_start(q_dram.opt()[c * P:(c + 1) * P, :],
                                      qt[:])
                    qsq = sbp.tile([P, TL], f32r, tag="sq")
                    nc.scalar.activation(qsq[:], acc[:], AF.Square)
                    nc.tensor.matmul(qsumsq[:], ones_r[:], qsq[:],
                                     start=(c == 0), stop=(c == NC - 1))
                qscale = _rsqrt(nc, scp, qsumsq, 1.0 / QKV, const["epsb"][:],
                                "scale")

                # ---- phase B per group: carried scans + y ----
                for g in range(2):
                    cwv_raw = ccp.tile([P, 8], f32, tag="cwvr")
                    nc.sync.dma_start(
                        cwv_raw[:], ag_outs[g].opt()[0:1, 0:1024]
                        .rearrange("o (c p) -> p (o c)", p=P))
                    cw_raw = ccp.tile([P, 8], f32, tag="cwr")
                    nc.sync.dma_start(
                        cw_raw[:], ag_outs[g].opt()[0:1, 1024:2048]
                        .rearrange("o (c p) -> p (o c)", p=P))
                    cwv_g = ccp.tile([P, 8], f32, tag="cwvg")
                    nc.vector.tensor_scalar(cwv_g[:], cwv_raw[:],
                                            gate_col[:], None, ALU.mult)
                    cw_g = ccp.tile([P, 8], f32, tag="cwg")
                    nc.vector.tensor_scalar(cw_g[:], cw_raw[:],
                                            gate_col[:], None, ALU.mult)
                    for c in range(8 * g, 8 * g + 8):
                        j = c - 8 * g
                        sw = pbp.tile([P, TL], f32, tag="sw")
                        nc.vector.tensor_tensor_scan(
                            sw[:], wv_t[c][:], wv_t[c][:], cwv_g[:, j:j + 1],
                            ALU.add, ALU.bypass)
                        sw2 = pbp.tile([P, TL], f32, tag="sw2")
                        nc.vector.tensor_tensor_scan(
                            sw2[:], w_t[c][:], w_t[c][:], cw_g[:, j:j + 1],
                            ALU.add, ALU.bypass)
                        qrb = pbp.tile([P, TL], f32, tag="qrb")
                        nc.sync.dma_start(
                            qrb[:], q_dram.opt()[c * P:(c + 1) * P, :])
                        qn = pbp.tile([P, TL], f32, tag="qn")
                        nc.vector.tensor_tensor(qn[:], qrb[:], qscale[:],
                                                ALU.mult)
                        et = pbp.tile([P, TL], f32, tag="et")
                        nc.scalar.activation(et[:], qn[:], AF.Exp, scale=-1.0)
                        den = pbp.tile([P, TL], f32, tag="den")
                        nc.gpsimd.tensor_scalar(den[:], sw2[:], 1e-6, None,
                                                ALU.add)
                        dd = pbp.tile([P, TL], f32, tag="dd")
                        nc.vector.scalar_tensor_tensor(
                            dd[:], et[:], 1.0, den[:], ALU.add, ALU.mult)
                        rr = pbp.tile([P, TL], f32, tag="rr")
                        nc.vector.reciprocal(rr[:], dd[:])
                        yf = yfp.tile([P, TL], f32r, tag="yf")
                        nc.vector.tensor_tensor(yf[:], sw[:], rr[:], ALU.mult)
                        yf_t[c] = yf

        # ---- swiglu ----
        with (
            tc.tile_pool(name="a_w16", bufs=2) as wp16,
            tc.tile_pool(name="a_u", bufs=NE) as up,
            tc.tile_pool(name="a_mt", bufs=NE) as mtp,
            tc.tile_pool(name="a_t2", bufs=2) as sb2,
        ):
            u_sb = [None] * NE
            m_t = [None] * NE
            for m in range(2 * E // P):
                wt = wp16.tile([P, NC, P], f32r, tag="wk16")
                nc.sync.dma_start(
                    wt[:], wswiT.ap()[m * P:(m + 1) * P, :]
                    .rearrange("p (a n) -> p a n", n=P))
                acc = ps.tile([P, TL], f32, tag="mm")
                for c in range(NC):
                    nc.tensor.matmul(acc[:], wt[:, c, :], yf_t[c][:],
                                     start=(c == 0), stop=(c == NC - 1))
                if m < NE:
                    ut = up.tile([P, TL], f32, tag="u")
                    nc.scalar.copy(ut[:], acc[:])
                    u_sb[m] = ut
                else:
                    c = m - NE
                    eg = sb2.tile([P, TL], f32, tag="eg")
                    nc.scalar.activation(eg[:], acc[:], AF.Exp, scale=-1.0)
                    gt = sb2.tile([P, TL], f32, tag="g")
                    nc.scalar.copy(gt[:], acc[:])
                    p_ug = sb2.tile([P, TL], f32, tag="p_ug")
                    nc.gpsimd.tensor_tensor(p_ug[:], u_sb[c][:], gt[:],
                                            ALU.mult)
                    dd = sb2.tile([P, TL], f32, tag="dd2")
                    nc.gpsimd.tensor_scalar(dd[:], eg[:], 1.0, None, ALU.add)
                    rr = sb2.tile([P, TL], f32, tag="rr2")
                    nc.vector.reciprocal(rr[:], dd[:])
                    mt = mtp.tile([P, TL], f32r, tag="mt")
                    nc.vector.tensor_tensor(mt[:], p_ug[:], rr[:], ALU.mult)
                    m_t[c] = mt

            # ---- out-proj + residual -> DRAM ----
            with tc.tile_pool(name="a_w8b", bufs=2) as wpb:
                for mo in range(NE):
                    wt = _wtile8(nc, wpb, woutT, mo)
                    acc = ps.tile([P, TL], f32, tag="mm")
                    for c in range(NE):
                        nc.tensor.matmul(acc[:], wt[:, c, :], m_t[c][:],
                                         start=(c == 0), stop=(c == NE - 1))
                    xr = sb2.tile([P, TL], f32, tag="xr")
                    nc.sync.dma_start(xr[:], x_ap[mo * P:(mo + 1) * P, :])
                    xo = sb2.tile([P, TL], f32, tag="xo")
                    nc.vector.tensor_tensor(xo[:], acc[:], xr[:], ALU.add)
                    nc.sync.dma_start(xout[mo * P:(mo + 1) * P, :], xo[:])


def _build_tea(tc, const, wqk4c, wv4c, wswiT4c, woutT4,
               agx_in, agx_out_h, rs_in, rs_out_h, outT):
    nc = tc.nc
    ones_r = const["ones_r"]
    cc_t, ss_t, cm_t = const["cc"], const["ss"], const["cmask"]
    HL = 8

    for half in range(2):
        nc.gpsimd.collective_compute(
            "AllGather", ALU.bypass, replica_groups=PAIRS,
            ins=[agx_in.opt()[half * (E // 2):(half + 1) * (E // 2), :]],
            outs=[agx_out_h[half].opt()])

    with (
        tc.tile_pool(name="t_yt", bufs=2 * HL) as ytp,       # 32 KB
        tc.tile_pool(name="t_sc", bufs=2) as scp,
        tc.tile_pool(name="t_ps", bufs=2, space="PSUM") as ps,
        tc.tile_pool(name="t_ps2", bufs=2, space="PSUM") as ps2,
        tc.tile_pool(name="t_xn", bufs=2 * NE) as xnp,       # 32 KB
        tc.tile_pool(name="t_v", bufs=16) as vp,             # 32 KB
    ):
        with tc.tile_pool(name="t_t", bufs=3) as sbp:
            # ---- rms(x3) ----
            xn = [[None] * NE for _ in range(2)]
            for tch in range(2):
                sumsq = ps2.tile([P, TL], f32, tag="sumsq")
                def _x3_ap(tch, e):
                    half, er = e // 4, e % 4
                    return agx_out_h[half].opt()[
                        tch * (E // 2) + er * P:tch * (E // 2) + (er + 1) * P, :]

                for e in range(NE):
                    xt3 = sbp.tile([P, TL], f32, tag="xt3")
                    nc.sync.dma_start(xt3[:], _x3_ap(tch, e))
                    xsq = sbp.tile([P, TL], f32r, tag="sq")
                    nc.scalar.activation(xsq[:], xt3[:], AF.Square)
                    nc.tensor.matmul(sumsq[:], ones_r[:], xsq[:],
                                     start=(e == 0), stop=(e == NE - 1))
                xscale = _rsqrt(nc, scp, sumsq, 1.0 / E, const["epsb"][:],
                                "xscale", ln_bufs=2)
                for e in range(NE):
                    xt3 = sbp.tile([P, TL], f32, tag="xt3")
                    nc.sync.dma_start(xt3[:], _x3_ap(tch, e))
                    t = xnp.tile([P, TL], f32r, tag="xn")
                    nc.vector.tensor_tensor(t[:], xt3[:], xscale[:],
                                            ALU.mult)
                    xn[tch][e] = t

            # ---- V (token-major) ----
            V = [[None] * 2 for _ in range(8)]
            with tc.tile_pool(name="t_vw", bufs=2) as vwp:
                for vb in range(2):
                    vw = vwp.tile([P, NE, TL], f32r, tag="vw")
                    nc.sync.dma_start(
                        vw[:],
                        wv4c.ap()[vb * P:(vb + 1) * P, :]
                        .rearrange("p (a n) -> p a n", n=TL))
                    for ttile in range(8):
                        tch, toff = ttile // 4, (ttile % 4) * P
                        acc = ps.tile([P, TL], f32, tag="mm")
                        for e in range(NE):
                            nc.tensor.matmul(
                                acc[:], xn[tch][e][:, toff:toff + P],
                                vw[:, e, :],
                                start=(e == 0), stop=(e == NE - 1))
                        vt = vp.tile([P, TL], f32r, tag="V")
                        nc.scalar.copy(vt[:], acc[:])
                        V[ttile][vb] = vt

        # ---- per-head rope/rms + attention ----
        yT = [[None] * 2 for _ in range(HL)]
        with (
            tc.tile_pool(name="t_qk", bufs=6) as qkp,
            tc.tile_pool(name="t_es", bufs=8) as esp,
            tc.tile_pool(name="t_w8", bufs=3) as wp,
            tc.tile_pool(name="t_at", bufs=2) as sba,
            tc.tile_pool(name="t_psa", bufs=2, space="PSUM") as psa,
            tc.tile_pool(name="t_psd", bufs=1, space="PSUM") as psd,
        ):
            sel4 = const["sel4"]
            for h in range(HL):
                qn_h = [None] * 2
                kn_h = [None] * 2
                sites = []
                coll = scp.tile([4, TL], f32, tag="coll", bufs=2)
                # pass 1: matmuls, (1xTL) sumsq rows into the collector, rope
                for wi, (which, mti, out_list) in enumerate(
                        (("q", h, qn_h), ("k", NE + h, kn_h))):
                    wt = _wtile8(nc, wp, wqk4c, mti)
                    for tch in range(2):
                        acc = ps.tile([P, TL], f32, tag="mm")
                        for e in range(NE):
                            nc.tensor.matmul(acc[:], wt[:, e, :],
                                             xn[tch][e][:],
                                             start=(e == 0),
                                             stop=(e == NE - 1))
                        zsq = sba.tile([P, TL], f32r, tag="sq")
                        nc.scalar.activation(zsq[:], acc[:], AF.Square)
                        sq_ps = ps2.tile([1, TL], f32, tag="sumsq")
                        nc.tensor.matmul(sq_ps[:], ones_r[:, 0:1], zsq[:],
                                         start=True, stop=True)
                        r = 2 * wi + tch
                        srow = scp.tile([1, TL], f32, tag="srow", bufs=3)
                        nc.scalar.copy(srow[:], sq_ps[:])
                        nc.sync.dma_start(coll[r:r + 1, :], srow[:])
                        tsl = slice(tch * TL, (tch + 1) * TL)
                        tmp1 = sba.tile([P, TL], f32, tag="tmp1")
                        nc.vector.tensor_tensor(tmp1[:], acc[:],
                                                cc_t[:, tsl], ALU.mult)
                        cross = sba.tile([P, TL], f32, tag="cross")
                        nc.vector.tensor_tensor(cross[:64, :], acc[64:, :],
                                                ss_t[:64, tsl], ALU.mult)
                        nc.vector.tensor_tensor(cross[64:, :], acc[:64, :],
                                                ss_t[64:, tsl], ALU.mult)
                        zrope = sba.tile([P, TL], f32, tag="zrope",
                                         bufs=3)
                        nc.gpsimd.tensor_tensor(zrope[:], tmp1[:], cross[:],
                                                ALU.add)
                        sites.append((r, zrope, out_list, tch))
                # one Ln + one Exp for all 4 sites of this head.
                # q rows: rsqrt(ssq + DH*eps); k rows: sqrt(DH)*that
                # (Exp bias = 0.5*ln(DH) on k rows).
                lnc = scp.tile([4, TL], f32, tag="lnc", bufs=2)
                nc.scalar.activation(lnc[:], coll[:], AF.Ln,
                                     bias=const["epsbdh"][0:4, :])
                esc = scp.tile([4, TL], f32r, tag="esc", bufs=2)
                nc.scalar.activation(esc[:], lnc[:], AF.Exp, scale=-0.5,
                                     bias=const["klnb"][:])
                for r, zrope, out_list, tch in sites:
                    sc_ps = ps.tile([P, TL], f32, tag="mm")
                    nc.tensor.matmul(sc_ps[:], sel4[:, r * P:(r + 1) * P],
                                     esc[:], start=True, stop=True)
                    zn = qkp.tile([P, TL], f32r, tag="zn")
                    nc.vector.tensor_tensor(zn[:], zrope[:], sc_ps[:],
                                            ALU.mult)
                    out_list[tch] = zn

                for qc in range(2):
                    denom = psd.tile([P, TL], f32, tag="denom")
                    ytil = psd.tile([P, TL], f32, tag="ytil")
                    nkt = 4 * (qc + 1)
                    for kt in range(nkt):
                        tch_k, koff = kt // 4, (kt % 4) * P
                        sT = psa.tile([P, TL], f32, tag="sT")
                        nc.tensor.matmul(sT[:],
                                         kn_h[tch_k][:, koff:koff + P],
                                         qn_h[qc][:], start=True, stop=True)
                        es = esp.tile([P, TL], f32r, tag="es")
                        j = kt - 4 * qc
                        if j >= 0:
                            sm = sba.tile([P, TL], f32, tag="sm")
                            nc.vector.tensor_tensor(
                                sm[:], sT[:], cm_t[:, j * TL:(j + 1) * TL],
                                ALU.add)
                            nc.scalar.activation(es[:], sm[:], AF.Exp)
                        else:
                            nc.scalar.activation(es[:], sT[:], AF.Exp)
                        nc.tensor.matmul(denom[:], ones_r[:], es[:],
                                         start=(kt == 0),
                                         stop=(kt == nkt - 1))
                        nc.tensor.matmul(
                            ytil[:],
                            V[kt][h // 4][:, (h % 4) * P:(h % 4 + 1) * P],
                            es[:], start=(kt == 0), stop=(kt == nkt - 1))
                    rr = sba.tile([P, TL], f32, tag="arr")
                    nc.vector.reciprocal(rr[:], denom[:])
                    yt = ytp.tile([P, TL], f32r, tag="yT")
                    nc.vector.tensor_tensor(yt[:], ytil[:], rr[:], ALU.mult)
                    yT[h][qc] = yt

        # ---- partial swiglu (all T tokens, my y channels) ----
        rs_in_h = rs_in
        with (
            tc.tile_pool(name="t_w8s", bufs=3) as wps,
            tc.tile_pool(name="t_pug", bufs=4) as pugp,
        ):
            # rs half h holds m-tiles [4h..4h+4) (u) and [8+4h..8+4h+4) (g)
            def _rs_slot(m):
                h = (m % NE) // 4
                loc = (m % 4) + 4 * (m // NE)   # 0..7 within half
                return h, loc

            for h in range(2):
                for m in list(range(4 * h, 4 * h + 4)) +                          list(range(NE + 4 * h, NE + 4 * h + 4)):
                    wt = _wtile8(nc, wps, wswiT4c, m)
                    _, loc = _rs_slot(m)
                    for tch in range(2):
                        acc = ps.tile([P, TL], f32, tag="mm")
                        for kk in range(HL):
                            nc.tensor.matmul(acc[:], wt[:, kk, :],
                                             yT[kk][tch][:],
                                             start=(kk == 0),
                                             stop=(kk == HL - 1))
                        pug = pugp.tile([P, TL], f32, tag="pug")
                        nc.scalar.copy(pug[:], acc[:])
                        nc.sync.dma_start(
                            rs_in_h[h].opt()[tch * E + loc * P:
                                             tch * E + (loc + 1) * P, :],
                            pug[:])
                nc.gpsimd.collective_compute(
                    "ReduceScatter", ALU.add, replica_groups=PAIRS,
                    ins=[rs_in_h[h].opt()], outs=[rs_out_h[h].opt()])

        # ---- silu + out-proj + residual ----
        with (
            tc.tile_pool(name="t_mt", bufs=NE) as mtp,
            tc.tile_pool(name="t_w8o", bufs=3) as wpo,
            tc.tile_pool(name="t_t4", bufs=2) as sb4,
        ):
            m_t = [None] * NE
            for c in range(NE):
                h, cr = c // 4, c % 4
                ut = sb4.tile([P, TL], f32, tag="u4")
                nc.sync.dma_start(
                    ut[:], rs_out_h[h].opt()[cr * P:(cr + 1) * P, :])
                gt = sb4.tile([P, TL], f32, tag="g4")
                nc.sync.dma_start(
                    gt[:],
                    rs_out_h[h].opt()[TL + cr * P:TL + (cr + 1) * P, :])
                eg = sb4.tile([P, TL], f32, tag="eg4")
                nc.scalar.activation(eg[:], gt[:], AF.Exp, scale=-1.0)
                p_ug = sb4.tile([P, TL], f32, tag="pug4")
                nc.gpsimd.tensor_tensor(p_ug[:], ut[:], gt[:], ALU.mult)
                dd = sb4.tile([P, TL], f32, tag="dd4")
                nc.gpsimd.tensor_scalar(dd[:], eg[:], 1.0, None, ALU.add)
                rr = sb4.tile([P, TL], f32, tag="rr4")
                nc.vector.reciprocal(rr[:], dd[:])
                mt = mtp.tile([P, TL], f32r, tag="mt4")
                nc.vector.tensor_tensor(mt[:], p_ug[:], rr[:], ALU.mult)
                m_t[c] = mt
            for mo in range(NE):
                wt = _wtile8(nc, wpo, woutT4, mo)
                acc = ps.tile([P, TL], f32, tag="mm")
                for c in range(NE):
                    nc.tensor.matmul(acc[:], wt[:, c, :], m_t[c][:],
                                     start=(c == 0), stop=(c == NE - 1))
                xr = sb4.tile([P, TL], f32, tag="xr4")
                nc.sync.dma_start(xr[:], agx_in.opt()[mo * P:(mo + 1) * P, :])
                xo = sb4.tile([P, TL], f32, tag="xo4")
                nc.vector.tensor_tensor(xo[:], acc[:], xr[:], ALU.add)
                nc.sync.dma_start(outT.ap()[mo * P:(mo + 1) * P, :], xo[:])


def build_program():
    nc = bacc.Bacc("TRN2", target_bir_lowering=False, debug=False,
                   num_devices=NCORES)

    din = {}

    def inp(name, shape, dt):
        din[name] = nc.dram_tensor(name, list(shape), dt,
                                   kind="ExternalInput")
        return din[name]

    inp("xT0", (E, TL), f32)
    for l in (1, 2, 3):
        inp(f"wqkvT{l}", (3 * QKV, E), f32r)       # tile layout [m*P, K]
        inp(f"wswiT{l}", (2 * E, QKV), f32r)
        inp(f"woutT{l}", (E, E), f32r)
    inp("wqk4c", (QKV, E), f32r)                   # [q_h0..q_h7, k_h0..k_h7]
    inp("wv4c", (2 * P, NE * TL), f32r)            # [vb, p, e, n]
    inp("wswiT4c", (QKV, E), f32r)
    inp("woutT4", (E, E), f32r)
    inp("cc", (P, T), f32)
    inp("ss", (P, T), f32)
    inp("cmask", (P, 4 * TL), f32)
    inp("gate", (P, 1), f32)
    inp("ones_r", (P, P), f32r)
    inp("sel4", (4, 4 * P), f32r)
    inp("klnb", (4, 1), f32)
    outT = nc.dram_tensor("outT", [E, TL], f32, kind="ExternalOutput")

    with tile.TileContext(nc) as tc:
        with (
            tc.tile_pool(name="const", bufs=1) as constp,
            tc.tile_pool(name="dram", bufs=1, space="DRAM") as dram,
        ):
            const = {}
            epsb = constp.tile([P, 1], f32, tag="epsb")
            nc.any.memset(epsb[:], EPS)
            const["epsb"] = epsb
            epsbdh = constp.tile([P, 1], f32, tag="epsbdh")
            nc.any.memset(epsbdh[:], DH * EPS)
            const["epsbdh"] = epsbdh
            for nm, dt in (("cc", f32), ("ss", f32), ("cmask", f32),
                           ("gate", f32), ("ones_r", f32r),
                           ("sel4", f32r), ("klnb", f32)):
                t = constp.tile(list(din[nm].shape), dt, tag=nm)
                nc.sync.dma_start(t[:], din[nm].ap())
                const[nm] = t

            xst = [None]
            for l in (1, 2, 3):
                xst.append(dram.tile([E, TL], f32, tag=f"xst{l}", name=f"xst{l}"))
            agx_in = xst[3]  # layer-3 output doubles as the TEA x3 AG input

            x_aps = [din["xT0"].ap(), xst[1].opt(), xst[2].opt()]
            for l in (1, 2, 3):
                ag_ins = [dram.tile([1, 2048], f32, tag=f"agi{l}_{g}",
                                    name=f"agi{l}_{g}") for g in range(2)]
                ag_outs = [dram.tile([2, 2048], f32, tag=f"ago{l}_{g}",
                                     name=f"ago{l}_{g}") for g in range(2)]
                q_dram = dram.tile([QKV, TL], f32, tag=f"qd{l}")
                _build_aft_layer(tc, const, x_aps[l - 1],
                                 din[f"wqkvT{l}"], din[f"wswiT{l}"],
                                 din[f"woutT{l}"], ag_ins, ag_outs, q_dram,
                                 xst[l].opt())

            agx_out_h = [dram.tile([E, TL], f32, tag=f"agxo{h}",
                                   name=f"agxo{h}") for h in range(2)]
            rs_in_h = [dram.tile([QKV, TL], f32, tag=f"rsi{h}",
                                 name=f"rsi{h}") for h in range(2)]
            rs_out_h = [dram.tile([E, TL], f32, tag=f"rso{h}",
                                  name=f"rso{h}") for h in range(2)]
            _build_tea(tc, const, din["wqk4c"], din["wv4c"],
                       din["wswiT4c"], din["woutT4"], agx_in, agx_out_h,
                       rs_in_h, rs_out_h, outT)

    nc.compile()
    return nc


# --------------------------------------------------------------------------
# host-side sharding
# --------------------------------------------------------------------------

def _host_inputs(inputs):
    f = np.float32
    cos = np.ascontiguousarray(np.asarray(inputs['cos'], f)[:, 0, :].T)
    sin = np.ascontiguousarray(np.asarray(inputs['sin'], f)[:, 0, :].T)
    cc = np.concatenate([cos, cos], 0)
    ss = np.concatenate([sin, -sin], 0)
    cm = np.zeros((4, P, TL), f)
    kk = np.arange(P)[:, None]
    qq = np.arange(TL)[None, :]
    for j in range(4):
        cm[j] = np.where(P * j + kk <= qq, 0.0, -1e30)
    cmask = np.ascontiguousarray(cm.transpose(1, 0, 2).reshape(P, 4 * TL))
    ones_r = np.ones((P, P), f)

    def tl(wT):
        # (K, M) -> tile layout (M, K): row-block m = [p, e, n] contiguous
        K, M = wT.shape
        return np.ascontiguousarray(
            wT.reshape(K // P, P, M // P, P).transpose(2, 1, 0, 3)
            .reshape(M, K))

    sel4 = np.zeros((4, 4 * P), f)
    for i in range(4):
        sel4[i, i * P:(i + 1) * P] = 1.0
    klnb = np.array([[0.0], [0.0], [0.5 * np.log(DH)], [0.5 * np.log(DH)]], f)
    shared = {'cc': cc, 'ss': ss, 'cmask': cmask, 'ones_r': ones_r,
              'sel4': sel4, 'klnb': klnb}
    for l in (1, 2, 3):
        shared[f'wqkvT{l}'] = tl(np.asarray(inputs[f'w_qkv{l}'], f).T)
        shared[f'wswiT{l}'] = tl(np.asarray(inputs[f'w_swiglu{l}'], f).T)
        shared[f'woutT{l}'] = tl(np.asarray(inputs[f'w_out{l}'], f).T)
    shared['woutT4'] = tl(np.asarray(inputs['w_out4'], f).T)

    wq4 = np.asarray(inputs['w_qkv4'], f).T       # (E, 6144): per-head blocks
    wswi4 = np.asarray(inputs['w_swiglu4'], f).T  # (QKV, 2E)
    by_par = {}
    for par in range(2):
        hs = par * 8
        qk_cols = []
        for part in range(2):   # q then k blocks
            for h in range(hs, hs + 8):
                qk_cols.append(wq4[:, h * 3 * DH + part * DH:
                                   h * 3 * DH + (part + 1) * DH])
        v_cols = [wq4[:, h * 3 * DH + 2 * DH: h * 3 * DH + 3 * DH]
                  for h in range(hs, hs + 8)]
        kv = np.concatenate(v_cols, 1)             # (E, 1024)
        # wv4c layout [vb, p, e, n]: element = kv[128e + p, vb*512 + n]
        wv4c = np.ascontiguousarray(
            kv.reshape(NE, P, 2, TL).transpose(2, 1, 0, 3)
            .reshape(2 * P, NE * TL))
        by_par[par] = {
            'wqk4c': tl(np.concatenate(qk_cols, 1)),
            'wv4c': wv4c,
            'wswiT4c': tl(np.ascontiguousarray(
                wswi4[hs * DH:(hs + 8) * DH, :])),
            'gate': np.full((P, 1), float(par), f),
        }

    x = np.asarray(inputs['x'], f)
    in_maps = []
    for c in range(NCORES):
        b, par = c // 2, c % 2
        m = dict(shared)
        m.update(by_par[par])
        m['xT0'] = np.ascontiguousarray(x[b, par * TL:(par + 1) * TL, :].T)
        in_maps.append(m)
    return in_maps


_cached = {}


def kernel(**inputs):
    if 'nc' not in _cached:
        _cached['nc'] = build_program()
    nc = _cached['nc']
    in_maps = _host_inputs(inputs)
    trace = bool(int(os.environ.get('BASS_KERNEL_TRACE', '0')))
    res = run_bass_kernel_spmd(nc, in_maps, core_ids=list(range(NCORES)),
                               trace=trace)
    _cached['last_results'] = res
    out = np.zeros((4, T, E), np.float32)
    for c in range(NCORES):
        b, par = c // 2, c % 2
        out[b, par * TL:(par + 1) * TL, :] = res.results[c]['outT'].T
    return out



# revision 5
# speedup vs baseline: 1.7896x; 1.7896x over previous
"""TRN2 Bass kernel for nn_Block_82325933129820.

3x AFT blocks + 1 transformer (TEA) block, B=4 T=1024 E=1024 QKV=2048 H=16.

Sharding: 8 cores = 4 batch-pairs. Within a pair (even core, odd core):
  - AFT layers: token-split (even: tokens 0-511, odd: 512-1023), feature-major
    activations (channels on partitions, tokens on free dim). The cumsum runs
    as per-chunk tensor_tensor_scan along the free dim; cross-core carries
    travel via pair AllGathers and enter as the scan's `initial` value, gated
    to zero on even cores (with the denominator's +1e-6 folded in).
  - TEA: head-split (even: heads 0-7, odd: 8-15) over the full 1024 tokens.
    x3 is pair-AllGathered in bf16; attention is computed in S^T layout; the
    swiglu partial contraction is pair-ReduceScattered in bf16.

Precision/perf strategy: all GEMM weights are bf16 (half the HBM traffic);
PSUM accumulation is fp32. AFT intermediate activations (q/k/w/wv/yf) are
bf16 which enables DVE 2x modes; the cumsum scan recurrence is fp32
internally regardless. TEA attention internals stay fp32 (f32r matmuls at
full PE rate for N>=512). All reciprocals run on the scalar engine as
Exp(-Ln(x)); rsqrt(x) = Exp(-0.5*Ln(x)); sigmoid/silu via Exp with the
reciprocal folded into existing products; "+1"/"+eps" constants folded into
activation bias / scan initials. The AFT swiglu's first 8 output tiles
accumulate c-interleaved with the cumsum pipeline so the tensor engine
stays busy through the vector-heavy phase.
"""
import os
import sys
import numpy as np
import ml_dtypes

for _p in ('/opt/trn_rl_repo',):
    if _p not in sys.path:
        sys.path.insert(0, _p)

import concourse.bass as bass
import concourse.mybir as mybir
import concourse.tile as tile
from concourse import bacc
from concourse.bass_utils import run_bass_kernel_spmd

P = 128
TL = 512          # AFT tokens per core
E = 1024
QKV = 2048
T = 1024
DH = 128
NCORES = 8
NE = E // P       # 8
NC = QKV // P     # 16
EPS = float(np.finfo(np.float32).eps)
f32 = mybir.dt.float32
f32r = mybir.dt.float32r
bf16 = mybir.dt.bfloat16
AF = mybir.ActivationFunctionType
ALU = mybir.AluOpType
PAIRS = [[0, 1], [2, 3], [4, 5], [6, 7]]
BF = ml_dtypes.bfloat16


def _rsqrt(nc, pool, src_ps, scale, bias_ap, tag, ln_bufs=None):
    """rsqrt(src*scale + bias) = Exp(-0.5*Ln(.)). src_ps is PSUM (P, n)."""
    n = src_ps.shape[-1]
    tmp = pool.tile([P, n], f32, tag="lntmp", bufs=ln_bufs)
    nc.scalar.activation(tmp[:], src_ps[:], AF.Ln, scale=scale, bias=bias_ap)
    out = pool.tile([P, n], bf16, tag=tag)
    nc.scalar.activation(out[:], tmp[:], AF.Exp, scale=-0.5)
    return out


def _wtile8(nc, pool, wdram, m, tag="wk8"):
    """(P, 8, P) bf16 weight tile m from host-pretiled (M_total, K=8P) DRAM."""
    wt = pool.tile([P, NE, P], bf16, tag=tag)
    nc.sync.dma_start(wt[:], wdram.ap()[m * P:(m + 1) * P, :]
                      .rearrange("p (a n) -> p a n", n=P))
    return wt


def _build_aft_layer(tc, const, x_tiles, xp, wqkvT, wswiC, woutT,
                     ag_ins, ag_outs, x3_bf=None):
    """One AFT layer, fully SBUF-resident activations.

    x_tiles: list of 8 (P, TL) f32 SBUF tiles (residual stream).
    Returns the new list of 8 x tiles (allocated from xp).
    If x3_bf is given (layer 3), also writes the bf16 output to that DRAM AP.
    """
    nc = tc.nc
    ones_b = const["ones_b"]
    gate_col = const["gate"]

    with (
        tc.tile_pool(name="a_sc", bufs=2) as scp,
        tc.tile_pool(name="a_k", bufs=NC) as kp,
        tc.tile_pool(name="a_q", bufs=NC) as qp,
        tc.tile_pool(name="a_ww", bufs=NC) as wwp,
        tc.tile_pool(name="a_yf", bufs=NC) as yfp,
        tc.tile_pool(name="a_cc", bufs=8) as ccp,
        tc.tile_pool(name="a_xn", bufs=NE) as xnp,
    ):
        yf_t = [None] * NC
        w_t = [None] * NC
        wv_t = [None] * NC
        with (
            tc.tile_pool(name="a_w8", bufs=4) as wp,
            tc.tile_pool(name="a_ld", bufs=4) as sbp,
            tc.tile_pool(name="a_ps", bufs=4, space="PSUM") as ps,
            tc.tile_pool(name="a_ps2", bufs=1, space="PSUM") as ps2,
        ):
            # ---- rms(x) ----
            xsq = []
            for e in range(NE):
                t = sbp.tile([P, TL], bf16, tag="sq", bufs=NE)
                nc.gpsimd.tensor_tensor(t[:], x_tiles[e][:], x_tiles[e][:],
                                        ALU.mult)
                xsq.append(t)
            sumsq = ps2.tile([P, TL], f32, tag="xsumsq")
            for e in range(NE):
                nc.tensor.matmul(sumsq[:], ones_b[:], xsq[e][:],
                                 start=(e == 0), stop=(e == NE - 1))
            xscale = _rsqrt(nc, scp, sumsq, 1.0 / E, const["epsb"][:],
                            "scale")
            xn = []
            for e in range(NE):
                t = xnp.tile([P, TL], bf16, tag="xn")
                nc.vector.tensor_tensor(t[:], x_tiles[e][:], xscale[:],
                                        ALU.mult)
                xn.append(t)

            def qkv_mtile(m):
                wt = _wtile8(nc, wp, wqkvT, m)
                acc = ps.tile([P, TL], f32, tag="mm")
                for e in range(NE):
                    nc.tensor.matmul(acc[:], wt[:, e, :], xn[e][:],
                                     start=(e == 0), stop=(e == NE - 1))
                return acc

            # ---- k tiles (SBUF-resident bf16) ----
            k_sb = [None] * NC
            ksq = [None] * NC
            for c in range(NC):
                acc = qkv_mtile(16 + c)
                kt = kp.tile([P, TL], bf16, tag="k")
                nc.scalar.copy(kt[:], acc[:])
                k_sb[c] = kt
                sq = sbp.tile([P, TL], bf16, tag="sq", bufs=NE)
                nc.gpsimd.tensor_tensor(sq[:], kt[:], kt[:], ALU.mult)
                ksq[c] = sq
            ksumsq = ps2.tile([P, TL], f32, tag="ksumsq")
            for c in range(NC):
                nc.tensor.matmul(ksumsq[:], ones_b[:], ksq[c][:],
                                 start=(c == 0), stop=(c == NC - 1))
            kscale = _rsqrt(nc, scp, ksumsq, 1.0 / QKV, const["epsb"][:],
                            "scale")

            # ---- v matmuls + w/wv + carries (2 groups of 8) ----
            for g in range(2):
                for c in range(8 * g, 8 * g + 8):
                    kn = sbp.tile([P, TL], bf16, tag="kn", bufs=3)
                    nc.vector.tensor_tensor(kn[:], k_sb[c][:], kscale[:],
                                            ALU.mult)
                    w = wwp.tile([P, TL], bf16, tag="w")
                    cw_col = ccp.tile([P, 1], f32, tag="cwc")
                    nc.scalar.activation(w[:], kn[:], AF.Exp,
                                         accum_out=cw_col[:])
                    acc = qkv_mtile(32 + c)
                    wv = wwp.tile([P, TL], bf16, tag="wv")
                    cwv_col = ccp.tile([P, 1], f32, tag="cwvc")
                    nc.vector.scalar_tensor_tensor(
                        wv[:], acc[:], 0.0, w[:], ALU.bypass, ALU.mult,
                        accum_out=cwv_col[:])
                    j = c - 8 * g
                    nc.sync.dma_start(
                        ag_ins[g].opt()[:, j * P:(j + 1) * P]
                        .rearrange("o (p q) -> p (o q)", p=P),
                        cwv_col[:])
                    nc.sync.dma_start(
                        ag_ins[g].opt()[:, 1024 + j * P:1024 + (j + 1) * P]
                        .rearrange("o (p q) -> p (o q)", p=P),
                        cw_col[:])
                    w_t[c] = w
                    wv_t[c] = wv
                nc.gpsimd.collective_compute(
                    "AllGather", ALU.bypass, replica_groups=PAIRS,
                    ins=[ag_ins[g].opt()], outs=[ag_outs[g].opt()])

            # ---- q tiles (SBUF-resident bf16) ----
            q_sb = [None] * NC
            qsq = [None] * NC
            for c in range(NC):
                acc = qkv_mtile(c)
                qt = qp.tile([P, TL], bf16, tag="q")
                nc.scalar.copy(qt[:], acc[:])
                q_sb[c] = qt
                sq = sbp.tile([P, TL], bf16, tag="sq", bufs=NE)
                nc.gpsimd.tensor_tensor(sq[:], qt[:], qt[:], ALU.mult)
                qsq[c] = sq
            qsumsq = ps2.tile([P, TL], f32, tag="qsumsq")
            for c in range(NC):
                nc.tensor.matmul(qsumsq[:], ones_b[:], qsq[c][:],
                                 start=(c == 0), stop=(c == NC - 1))
            qscale = _rsqrt(nc, scp, qsumsq, 1.0 / QKV, const["epsb"][:],
                            "scale")

        # ---- phase B (scans etc.) interleaved with swiglu pass 1 ----
        with (
            tc.tile_pool(name="a_sw", bufs=4) as swp,
            tc.tile_pool(name="a_pb", bufs=3) as pbp,
            tc.tile_pool(name="a_u", bufs=NE) as up,
            tc.tile_pool(name="a_mt", bufs=NE) as mtp,
            tc.tile_pool(name="a_pss", bufs=8, space="PSUM") as pss,
        ):
            sacc = [None] * NE
            for g in range(2):
                cwv_raw = ccp.tile([P, 8], f32, tag="cwvr")
                nc.sync.dma_start(
                    cwv_raw[:], ag_outs[g].opt()[0:1, 0:1024]
                    .rearrange("o (c p) -> p (o c)", p=P))
                cw_raw = ccp.tile([P, 8], f32, tag="cwr")
                nc.sync.dma_start(
                    cw_raw[:], ag_outs[g].opt()[0:1, 1024:2048]
                    .rearrange("o (c p) -> p (o c)", p=P))
                cwv_g = ccp.tile([P, 8], f32, tag="cwvg")
                nc.vector.tensor_scalar(cwv_g[:], cwv_raw[:],
                                        gate_col[:], None, ALU.mult)
                # denominator carry gets the +1e-6 folded in
                cw_g = ccp.tile([P, 8], f32, tag="cwg")
                nc.vector.tensor_scalar(cw_g[:], cw_raw[:],
                                        gate_col[:], 1e-6,
                                        ALU.mult, ALU.add)
                for c in range(8 * g, 8 * g + 8):
                    j = c - 8 * g
                    sw = pbp.tile([P, TL], bf16, tag="sw")
                    nc.vector.tensor_tensor_scan(
                        sw[:], wv_t[c][:], wv_t[c][:], cwv_g[:, j:j + 1],
                        ALU.add, ALU.bypass)
                    sw2 = pbp.tile([P, TL], bf16, tag="sw2")
                    nc.vector.tensor_tensor_scan(
                        sw2[:], w_t[c][:], w_t[c][:], cw_g[:, j:j + 1],
                        ALU.add, ALU.bypass)
                    qn = pbp.tile([P, TL], bf16, tag="qn")
                    nc.gpsimd.tensor_tensor(qn[:], q_sb[c][:], qscale[:],
                                            ALU.mult)
                    et = pbp.tile([P, TL], bf16, tag="et")
                    nc.scalar.activation(et[:], qn[:], AF.Exp, scale=-1.0)
                    # dd = (et + 1) * sw2   (sw2 already carries the +1e-6)
                    dd = pbp.tile([P, TL], bf16, tag="dd")
                    nc.vector.scalar_tensor_tensor(
                        dd[:], et[:], 1.0, sw2[:], ALU.add, ALU.mult)
                    lnd = pbp.tile([P, TL], bf16, tag="lnd")
                    nc.scalar.activation(lnd[:], dd[:], AF.Ln)
                    rr = pbp.tile([P, TL], bf16, tag="rr")
                    nc.scalar.activation(rr[:], lnd[:], AF.Exp, scale=-1.0)
                    yf = yfp.tile([P, TL], bf16, tag="yf")
                    nc.vector.tensor_tensor(yf[:], sw[:], rr[:], ALU.mult)
                    yf_t[c] = yf
                    # swiglu pass 1 (u half, m=0..7), c-interleaved
                    wt = swp.tile([P, NE, P], bf16, tag="w1")
                    nc.sync.dma_start(
                        wt[:], wswiC.ap()[c * P:(c + 1) * P, 0:E]
                        .rearrange("p (a n) -> p a n", n=P))
                    for m in range(NE):
                        if c == 0:
                            sacc[m] = pss.tile([P, TL], f32, tag="sacc", name="sacc")
                        nc.tensor.matmul(sacc[m][:], wt[:, m, :], yf[:],
                                         start=(c == 0), stop=(c == NC - 1))

            # drain u, then swiglu pass 2 (g half, m=8..15), c-outer
            u_sb = [None] * NE
            for m in range(NE):
                ut = up.tile([P, TL], bf16, tag="u")
                nc.scalar.copy(ut[:], sacc[m][:])
                u_sb[m] = ut
            sacc2 = [None] * NE
            for c in range(NC):
                wt = swp.tile([P, NE, P], bf16, tag="w2")
                nc.sync.dma_start(
                    wt[:], wswiC.ap()[c * P:(c + 1) * P, E:2 * E]
                    .rearrange("p (a n) -> p a n", n=P))
                for m in range(NE):
                    if c == 0:
                        sacc2[m] = pss.tile([P, TL], f32, tag="sacc", name="sacc2")
                    nc.tensor.matmul(sacc2[m][:], wt[:, m, :], yf_t[c][:],
                                     start=(c == 0), stop=(c == NC - 1))
            # silu: m = u * g / (1 + exp(-g))
            m_t = [None] * NE
            for m in range(NE):
                eg = pbp.tile([P, TL], bf16, tag="eg")
                nc.scalar.activation(eg[:], sacc2[m][:], AF.Exp, scale=-1.0)
                lnd = pbp.tile([P, TL], bf16, tag="lnd")
                nc.scalar.activation(lnd[:], eg[:], AF.Ln,
                                     bias=const["oneb"][:])
                rr = pbp.tile([P, TL], bf16, tag="rr")
                nc.scalar.activation(rr[:], lnd[:], AF.Exp, scale=-1.0)
                pug = pbp.tile([P, TL], bf16, tag="pug")
                nc.vector.tensor_tensor(pug[:], u_sb[m][:], sacc2[m][:],
                                        ALU.mult)
                mt = mtp.tile([P, TL], bf16, tag="mt")
                nc.gpsimd.tensor_tensor(mt[:], pug[:], rr[:], ALU.mult)
                m_t[m] = mt

            # ---- out-proj + residual (SBUF resident) ----
            new_x = []
            with tc.tile_pool(name="a_w8b", bufs=3) as wpb:
                for mo in range(NE):
                    wt = _wtile8(nc, wpb, woutT, mo)
                    acc = pss.tile([P, TL], f32, tag="sacc")
                    for c in range(NE):
                        nc.tensor.matmul(acc[:], wt[:, c, :], m_t[c][:],
                                         start=(c == 0), stop=(c == NE - 1))
                    xo = xp.tile([P, TL], f32, tag="x", bufs=10)
                    nc.vector.tensor_tensor(xo[:], acc[:], x_tiles[mo][:],
                                            ALU.add)
                    new_x.append(xo)
                    if x3_bf is not None:
                        xob = pbp.tile([P, TL], bf16, tag="xob")
                        nc.scalar.copy(xob[:], xo[:])
                        nc.sync.dma_start(
                            x3_bf[mo * P:(mo + 1) * P, :], xob[:])
    return new_x


def _build_tea(tc, const, x_tiles, wqk4c, wv4c, wswiT4c, woutT4,
               agx_out_h, rs_in_h, rs_out_h, outT):
    nc = tc.nc
    ones_r = const["ones_r"]
    cc_t, ss_t, cm_t = const["cc"], const["ss"], const["cmask"]
    HL = 8

    with (
        tc.tile_pool(name="t_yt", bufs=2 * HL) as ytp,
        tc.tile_pool(name="t_sc", bufs=2) as scp,
        tc.tile_pool(name="t_ps", bufs=2, space="PSUM") as ps,
        tc.tile_pool(name="t_ps2", bufs=2, space="PSUM") as ps2,
        tc.tile_pool(name="t_xn", bufs=2 * NE) as xnp,
        tc.tile_pool(name="t_v", bufs=16) as vp,
    ):
        with tc.tile_pool(name="t_t", bufs=3) as sbp:
            # ---- rms(x3) (x3 arrives bf16 via the pair AllGather) ----
            xn = [[None] * NE for _ in range(2)]
            for tch in range(2):
                def _x3_ap(tch, e):
                    half, er = e // 4, e % 4
                    return agx_out_h[half].opt()[
                        tch * (E // 2) + er * P:tch * (E // 2) + (er + 1) * P, :]

                xt3s = []
                for e in range(NE):
                    xt3 = sbp.tile([P, TL], bf16, tag="xt3", bufs=NE)
                    nc.sync.dma_start(xt3[:], _x3_ap(tch, e))
                    xt3s.append(xt3)
                sumsq = ps2.tile([P, TL], f32, tag="sumsq")
                for e in range(NE):
                    xsq = sbp.tile([P, TL], bf16, tag="sq")
                    nc.gpsimd.tensor_tensor(xsq[:], xt3s[e][:], xt3s[e][:],
                                            ALU.mult)
                    nc.tensor.matmul(sumsq[:], const["ones_b"][:], xsq[:],
                                     start=(e == 0), stop=(e == NE - 1))
                xscale = _rsqrt(nc, scp, sumsq, 1.0 / E, const["epsb"][:],
                                "xscale", ln_bufs=2)
                for e in range(NE):
                    t = xnp.tile([P, TL], bf16, tag="xn")
                    nc.vector.tensor_tensor(t[:], xt3s[e][:], xscale[:],
                                            ALU.mult)
                    xn[tch][e] = t

            # ---- V (token-major) ----
            V = [[None] * 2 for _ in range(8)]
            with tc.tile_pool(name="t_vw", bufs=2) as vwp:
                for vb in range(2):
                    vw = vwp.tile([P, NE, TL], bf16, tag="vw")
                    nc.sync.dma_start(
                        vw[:],
                        wv4c.ap()[vb * P:(vb + 1) * P, :]
                        .rearrange("p (a n) -> p a n", n=TL))
                    for ttile in range(8):
                        tch, toff = ttile // 4, (ttile % 4) * P
                        acc = ps.tile([P, TL], f32, tag="mm")
                        for e in range(NE):
                            nc.tensor.matmul(
                                acc[:], xn[tch][e][:, toff:toff + P],
                                vw[:, e, :],
                                start=(e == 0), stop=(e == NE - 1))
                        vt = vp.tile([P, TL], bf16, tag="V")
                        nc.scalar.copy(vt[:], acc[:])
                        V[ttile][vb] = vt

        # ---- per-head rope/rms + attention ----
        yT = [[None] * 2 for _ in range(HL)]
        with (
            tc.tile_pool(name="t_qk", bufs=6) as qkp,
            tc.tile_pool(name="t_es", bufs=8) as esp,
            tc.tile_pool(name="t_w8", bufs=3) as wp,
            tc.tile_pool(name="t_at", bufs=2) as sba,
            tc.tile_pool(name="t_psa", bufs=2, space="PSUM") as psa,
            tc.tile_pool(name="t_psd", bufs=1, space="PSUM") as psd,
        ):
            sel4 = const["sel4"]
            for h in range(HL):
                qn_h = [None] * 2
                kn_h = [None] * 2
                sites = []
                coll = scp.tile([4, TL], f32, tag="coll", bufs=2)
                # pass 1: matmuls, (1xTL) sumsq rows into the collector, rope
                for wi, (which, mti, out_list) in enumerate(
                        (("q", h, qn_h), ("k", NE + h, kn_h))):
                    wt = _wtile8(nc, wp, wqk4c, mti)
                    for tch in range(2):
                        acc = ps.tile([P, TL], f32, tag="mm")
                        for e in range(NE):
                            nc.tensor.matmul(acc[:], wt[:, e, :],
                                             xn[tch][e][:],
                                             start=(e == 0),
                                             stop=(e == NE - 1))
                        zsq = sba.tile([P, TL], f32r, tag="sq")
                        nc.scalar.activation(zsq[:], acc[:], AF.Square)
                        sq_ps = ps2.tile([1, TL], f32, tag="sumsq")
                        nc.tensor.matmul(sq_ps[:], ones_r[:, 0:1], zsq[:],
                                         start=True, stop=True)
                        r = 2 * wi + tch
                        srow = scp.tile([1, TL], f32, tag="srow", bufs=3)
                        nc.scalar.copy(srow[:], sq_ps[:])
                        nc.sync.dma_start(coll[r:r + 1, :], srow[:])
                        tsl = slice(tch * TL, (tch + 1) * TL)
                        tmp1 = sba.tile([P, TL], f32, tag="tmp1")
                        nc.vector.tensor_tensor(tmp1[:], acc[:],
                                                cc_t[:, tsl], ALU.mult)
                        cross = sba.tile([P, TL], f32, tag="cross")
                        nc.vector.tensor_tensor(cross[:64, :], acc[64:, :],
                                                ss_t[:64, tsl], ALU.mult)
                        nc.vector.tensor_tensor(cross[64:, :], acc[:64, :],
                                                ss_t[64:, tsl], ALU.mult)
                        zrope = sba.tile([P, TL], f32, tag="zrope",
                                         bufs=3)
                        nc.gpsimd.tensor_tensor(zrope[:], tmp1[:], cross[:],
                                                ALU.add)
                        sites.append((r, zrope, out_list, tch))
                # one Ln + one Exp for all 4 sites of this head.
                lnc = scp.tile([4, TL], f32, tag="lnc", bufs=2)
                nc.scalar.activation(lnc[:], coll[:], AF.Ln,
                                     bias=const["epsbdh"][0:4, :])
                esc = scp.tile([4, TL], f32r, tag="esc", bufs=2)
                nc.scalar.activation(esc[:], lnc[:], AF.Exp, scale=-0.5,
                                     bias=const["klnb"][:])
                for r, zrope, out_list, tch in sites:
                    sc_ps = ps.tile([P, TL], f32, tag="mm")
                    nc.tensor.matmul(sc_ps[:], sel4[:, r * P:(r + 1) * P],
                                     esc[:], start=True, stop=True)
                    zn = qkp.tile([P, TL], f32r, tag="zn")
                    nc.vector.tensor_tensor(zn[:], zrope[:], sc_ps[:],
                                            ALU.mult)
                    out_list[tch] = zn

                for qc in range(2):
                    denom = psd.tile([P, TL], f32, tag="denom")
                    ytil = psd.tile([P, TL], f32, tag="ytil")
                    nkt = 4 * (qc + 1)
                    for kt in range(nkt):
                        tch_k, koff = kt // 4, (kt % 4) * P
                        sT = psa.tile([P, TL], f32, tag="sT")
                        nc.tensor.matmul(sT[:],
                                         kn_h[tch_k][:, koff:koff + P],
                                         qn_h[qc][:], start=True, stop=True)
                        es = esp.tile([P, TL], bf16, tag="es")
                        j = kt - 4 * qc
                        if j >= 0:
                            sm = sba.tile([P, TL], f32, tag="sm")
                            nc.vector.tensor_tensor(
                                sm[:], sT[:], cm_t[:, j * TL:(j + 1) * TL],
                                ALU.add)
                            nc.scalar.activation(es[:], sm[:], AF.Exp)
                        else:
                            nc.scalar.activation(es[:], sT[:], AF.Exp)
                        nc.tensor.matmul(denom[:], const["ones_b"][:], es[:],
                                         start=(kt == 0),
                                         stop=(kt == nkt - 1))
                        nc.tensor.matmul(
                            ytil[:],
                            V[kt][h // 4][:, (h % 4) * P:(h % 4 + 1) * P],
                            es[:], start=(kt == 0), stop=(kt == nkt - 1))
                    lnr = sba.tile([P, TL], f32, tag="lnr")
                    nc.scalar.activation(lnr[:], denom[:], AF.Ln)
                    rr = sba.tile([P, TL], f32, tag="arr")
                    nc.scalar.activation(rr[:], lnr[:], AF.Exp, scale=-1.0)
                    yt = ytp.tile([P, TL], bf16, tag="yT")
                    nc.vector.tensor_tensor(yt[:], ytil[:], rr[:], ALU.mult)
                    yT[h][qc] = yt

        # ---- partial swiglu (all T tokens, my y channels) ----
        with (
            tc.tile_pool(name="t_w8s", bufs=3) as wps,
            tc.tile_pool(name="t_pug", bufs=4) as pugp,
        ):
            # rs half h holds m-tiles [4h..4h+4) (u) and [8+4h..8+4h+4) (g)
            def _rs_slot(m):
                h = (m % NE) // 4
                loc = (m % 4) + 4 * (m // NE)   # 0..7 within half
                return h, loc

            for h in range(2):
                for m in list(range(4 * h, 4 * h + 4)) + \
                        list(range(NE + 4 * h, NE + 4 * h + 4)):
                    wt = _wtile8(nc, wps, wswiT4c, m)
                    _, loc = _rs_slot(m)
                    for tch in range(2):
                        acc = ps.tile([P, TL], f32, tag="mm")
                        for kk in range(HL):
                            nc.tensor.matmul(acc[:], wt[:, kk, :],
                                             yT[kk][tch][:],
                                             start=(kk == 0),
                                             stop=(kk == HL - 1))
                        pug = pugp.tile([P, TL], bf16, tag="pug")
                        nc.scalar.copy(pug[:], acc[:])
                        nc.sync.dma_start(
                            rs_in_h[h].opt()[tch * E + loc * P:
                                             tch * E + (loc + 1) * P, :],
                            pug[:])
                nc.gpsimd.collective_compute(
                    "ReduceScatter", ALU.add, replica_groups=PAIRS,
                    ins=[rs_in_h[h].opt()], outs=[rs_out_h[h].opt()])

        # ---- silu + out-proj + residual ----
        with (
            tc.tile_pool(name="t_mt", bufs=NE) as mtp,
            tc.tile_pool(name="t_w8o", bufs=3) as wpo,
            tc.tile_pool(name="t_t4", bufs=2) as sb4,
        ):
            m_t = [None] * NE
            for c in range(NE):
                h, cr = c // 4, c % 4
                ut = sb4.tile([P, TL], bf16, tag="u4")
                nc.sync.dma_start(
                    ut[:], rs_out_h[h].opt()[cr * P:(cr + 1) * P, :])
                gt = sb4.tile([P, TL], bf16, tag="g4")
                nc.sync.dma_start(
                    gt[:],
                    rs_out_h[h].opt()[TL + cr * P:TL + (cr + 1) * P, :])
                eg = sb4.tile([P, TL], bf16, tag="eg4")
                nc.scalar.activation(eg[:], gt[:], AF.Exp, scale=-1.0)
                lnd = sb4.tile([P, TL], bf16, tag="lnd4")
                nc.scalar.activation(lnd[:], eg[:], AF.Ln,
                                     bias=const["oneb"][:])
                rr = sb4.tile([P, TL], bf16, tag="rr4")
                nc.scalar.activation(rr[:], lnd[:], AF.Exp, scale=-1.0)
                pug = sb4.tile([P, TL], bf16, tag="pug4")
                nc.gpsimd.tensor_tensor(pug[:], ut[:], gt[:], ALU.mult)
                mt = mtp.tile([P, TL], bf16, tag="mt4")
                nc.vector.tensor_tensor(mt[:], pug[:], rr[:], ALU.mult)
                m_t[c] = mt
            for mo in range(NE):
                wt = _wtile8(nc, wpo, woutT4, mo)
                acc = ps.tile([P, TL], f32, tag="mm")
                for c in range(NE):
                    nc.tensor.matmul(acc[:], wt[:, c, :], m_t[c][:],
                                     start=(c == 0), stop=(c == NE - 1))
                xo = sb4.tile([P, TL], f32, tag="xo4")
                nc.vector.tensor_tensor(xo[:], acc[:], x_tiles[mo][:],
                                        ALU.add)
                nc.sync.dma_start(outT.ap()[mo * P:(mo + 1) * P, :], xo[:])


def build_program():
    nc = bacc.Bacc("TRN2", target_bir_lowering=False, debug=False,
                   num_devices=NCORES)

    din = {}

    def inp(name, shape, dt):
        din[name] = nc.dram_tensor(name, list(shape), dt,
                                   kind="ExternalInput")
        return din[name]

    inp("xT0", (E, TL), f32)
    for l in (1, 2, 3):
        inp(f"wqkvT{l}", (3 * QKV, E), bf16)       # tile layout [m*P, K]
        inp(f"wswiC{l}", (QKV, 2 * E), bf16)       # plain W.T (K rows, M cols)
        inp(f"woutT{l}", (E, E), bf16)
    inp("wqk4c", (QKV, E), bf16)                   # [q_h0..q_h7, k_h0..k_h7]
    inp("wv4c", (2 * P, NE * TL), bf16)            # [vb, p, e, n]
    inp("wswiT4c", (QKV, E), bf16)
    inp("woutT4", (E, E), bf16)
    inp("cc", (P, T), f32)
    inp("ss", (P, T), f32)
    inp("cmask", (P, 4 * TL), f32)
    inp("gate", (P, 1), f32)
    inp("ones_r", (P, P), f32r)
    inp("ones_b", (P, P), bf16)
    inp("sel4", (4, 4 * P), f32r)
    inp("klnb", (4, 1), f32)
    outT = nc.dram_tensor("outT", [E, TL], f32, kind="ExternalOutput")

    with tile.TileContext(nc) as tc:
        with (
            tc.tile_pool(name="const", bufs=1) as constp,
            tc.tile_pool(name="xres", bufs=10) as xp,
            tc.tile_pool(name="dram", bufs=1, space="DRAM") as dram,
        ):
            const = {}
            epsb = constp.tile([P, 1], f32, tag="epsb")
            nc.any.memset(epsb[:], EPS)
            const["epsb"] = epsb
            epsbdh = constp.tile([P, 1], f32, tag="epsbdh")
            nc.any.memset(epsbdh[:], DH * EPS)
            const["epsbdh"] = epsbdh
            oneb = constp.tile([P, 1], f32, tag="oneb")
            nc.any.memset(oneb[:], 1.0)
            const["oneb"] = oneb
            for nm, dt in (("cc", f32), ("ss", f32), ("cmask", f32),
                           ("gate", f32), ("ones_r", f32r),
                           ("ones_b", bf16), ("sel4", f32r), ("klnb", f32)):
                t = constp.tile(list(din[nm].shape), dt, tag=nm)
                nc.sync.dma_start(t[:], din[nm].ap())
                const[nm] = t

            # load residual stream into SBUF once
            x_tiles = []
            for e in range(NE):
                xt = xp.tile([P, TL], f32, tag="x", bufs=10)
                nc.sync.dma_start(xt[:], din["xT0"].ap()[e * P:(e + 1) * P, :])
                x_tiles.append(xt)

            agx_in = dram.tile([E, TL], bf16, tag="agx", name="agx")
            for l in (1, 2, 3):
                ag_ins = [dram.tile([1, 2048], f32, tag=f"agi{l}_{g}",
                                    name=f"agi{l}_{g}") for g in range(2)]
                ag_outs = [dram.tile([2, 2048], f32, tag=f"ago{l}_{g}",
                                     name=f"ago{l}_{g}") for g in range(2)]
                x_tiles = _build_aft_layer(
                    tc, const, x_tiles, xp,
                    din[f"wqkvT{l}"], din[f"wswiC{l}"], din[f"woutT{l}"],
                    ag_ins, ag_outs,
                    x3_bf=(agx_in.opt() if l == 3 else None))

            agx_out_h = [dram.tile([E, TL], bf16, tag=f"agxo{h}",
                                   name=f"agxo{h}") for h in range(2)]
            for half in range(2):
                nc.gpsimd.collective_compute(
                    "AllGather", ALU.bypass, replica_groups=PAIRS,
                    ins=[agx_in.opt()[half * (E // 2):(half + 1) * (E // 2), :]],
                    outs=[agx_out_h[half].opt()])
            rs_in_h = [dram.tile([QKV, TL], bf16, tag=f"rsi{h}",
                                 name=f"rsi{h}") for h in range(2)]
            rs_out_h = [dram.tile([E, TL], bf16, tag=f"rso{h}",
                                  name=f"rso{h}") for h in range(2)]
            _build_tea(tc, const, x_tiles, din["wqk4c"], din["wv4c"],
                       din["wswiT4c"], din["woutT4"], agx_out_h,
                       rs_in_h, rs_out_h, outT)

    nc.compile()
    return nc


# --------------------------------------------------------------------------
# host-side sharding
# --------------------------------------------------------------------------

def _host_inputs(inputs):
    f = np.float32
    cos = np.ascontiguousarray(np.asarray(inputs['cos'], f)[:, 0, :].T)
    sin = np.ascontiguousarray(np.asarray(inputs['sin'], f)[:, 0, :].T)
    cc = np.concatenate([cos, cos], 0)
    ss = np.concatenate([sin, -sin], 0)
    cm = np.zeros((4, P, TL), f)
    kk = np.arange(P)[:, None]
    qq = np.arange(TL)[None, :]
    for j in range(4):
        cm[j] = np.where(P * j + kk <= qq, 0.0, -1e30)
    cmask = np.ascontiguousarray(cm.transpose(1, 0, 2).reshape(P, 4 * TL))
    ones_r = np.ones((P, P), f)
    ones_b = np.ones((P, P), BF)

    def tl(wT):
        # (K, M) -> tile layout (M, K): row-block m = [p, e, n] contiguous
        K, M = wT.shape
        return np.ascontiguousarray(
            wT.reshape(K // P, P, M // P, P).transpose(2, 1, 0, 3)
            .reshape(M, K))

    sel4 = np.zeros((4, 4 * P), f)
    for i in range(4):
        sel4[i, i * P:(i + 1) * P] = 1.0
    klnb = np.array([[0.0], [0.0], [0.5 * np.log(DH)], [0.5 * np.log(DH)]], f)
    shared = {'cc': cc, 'ss': ss, 'cmask': cmask, 'ones_r': ones_r,
              'ones_b': ones_b, 'sel4': sel4, 'klnb': klnb}
    for l in (1, 2, 3):
        shared[f'wqkvT{l}'] = tl(np.asarray(inputs[f'w_qkv{l}'], f).T).astype(BF)
        shared[f'wswiC{l}'] = np.ascontiguousarray(
            np.asarray(inputs[f'w_swiglu{l}'], f).T).astype(BF)
        shared[f'woutT{l}'] = tl(np.asarray(inputs[f'w_out{l}'], f).T).astype(BF)
    shared['woutT4'] = tl(np.asarray(inputs['w_out4'], f).T).astype(BF)

    wq4 = np.asarray(inputs['w_qkv4'], f).T       # (E, 6144): per-head blocks
    wswi4 = np.asarray(inputs['w_swiglu4'], f).T  # (QKV, 2E)
    by_par = {}
    for par in range(2):
        hs = par * 8
        qk_cols = []
        for part in range(2):   # q then k blocks
            for h in range(hs, hs + 8):
                qk_cols.append(wq4[:, h * 3 * DH + part * DH:
                                   h * 3 * DH + (part + 1) * DH])
        v_cols = [wq4[:, h * 3 * DH + 2 * DH: h * 3 * DH + 3 * DH]
                  for h in range(hs, hs + 8)]
        kv = np.concatenate(v_cols, 1)             # (E, 1024)
        # wv4c layout [vb, p, e, n]: element = kv[128e + p, vb*512 + n]
        wv4c = np.ascontiguousarray(
            kv.reshape(NE, P, 2, TL).transpose(2, 1, 0, 3)
            .reshape(2 * P, NE * TL))
        by_par[par] = {
            'wqk4c': tl(np.concatenate(qk_cols, 1)).astype(BF),
            'wv4c': wv4c.astype(BF),
            'wswiT4c': tl(np.ascontiguousarray(
                wswi4[hs * DH:(hs + 8) * DH, :])).astype(BF),
            'gate': np.full((P, 1), float(par), f),
        }

    x = np.asarray(inputs['x'], f)
    in_maps = []
    for c in range(NCORES):
        b, par = c // 2, c % 2
        m = dict(shared)
        m.update(by_par[par])
        m['xT0'] = np.ascontiguousarray(x[b, par * TL:(par + 1) * TL, :].T)
        in_maps.append(m)
    return in_maps


_cached = {}


def kernel(**inputs):
    if 'nc' not in _cached:
        _cached['nc'] = build_program()
    nc = _cached['nc']
    in_maps = _host_inputs(inputs)
    trace = bool(int(os.environ.get('BASS_KERNEL_TRACE', '0')))
    res = run_bass_kernel_spmd(nc, in_maps, core_ids=list(range(NCORES)),
                               trace=trace)
    _cached['last_results'] = res
    out = np.zeros((4, T, E), np.float32)
    for c in range(NCORES):
        b, par = c // 2, c % 2
        out[b, par * TL:(par + 1) * TL, :] = res.results[c]['outT'].T
    return out


# revision 7
# speedup vs baseline: 2.0249x; 1.1315x over previous
"""TRN2 Bass kernel for nn_Block_82325933129820.

3x AFT blocks + 1 transformer (TEA) block, B=4 T=1024 E=1024 QKV=2048 H=16.

Sharding: 8 cores = 4 batch-pairs. Within a pair (even core, odd core):
  - AFT layers: token-split (even: tokens 0-511, odd: 512-1023), feature-major
    activations (channels on partitions, tokens on free dim). The cumsum runs
    as per-chunk tensor_tensor_scan along the free dim; cross-core carries
    travel via pair AllGathers and enter as the scan's `initial` value, gated
    to zero on even cores (with the denominator's +1e-6 folded in).
  - TEA: head-split (even: heads 0-7, odd: 8-15) over the full 1024 tokens.
    x3 is pair-AllGathered in bf16; attention is computed in S^T layout; the
    swiglu partial contraction is pair-ReduceScattered in bf16.

Precision/perf strategy: all GEMM weights are bf16 (half the HBM traffic);
PSUM accumulation is fp32. AFT intermediate activations (q/k/w/wv/yf) are
bf16 which enables DVE 2x modes; the cumsum scan recurrence is fp32
internally regardless. TEA attention internals stay fp32 (f32r matmuls at
full PE rate for N>=512). All reciprocals run on the scalar engine as
Exp(-Ln(x)); rsqrt(x) = Exp(-0.5*Ln(x)); sigmoid/silu via Exp with the
reciprocal folded into existing products; "+1"/"+eps" constants folded into
activation bias / scan initials. The AFT swiglu's first 8 output tiles
accumulate c-interleaved with the cumsum pipeline so the tensor engine
stays busy through the vector-heavy phase.
"""
import os
import sys
import numpy as np
import ml_dtypes

for _p in ('/opt/trn_rl_repo',):
    if _p not in sys.path:
        sys.path.insert(0, _p)

import concourse.bass as bass
import concourse.mybir as mybir
import concourse.tile as tile
from concourse import bacc
from concourse.bass_utils import run_bass_kernel_spmd

P = 128
TL = 512          # AFT tokens per core
E = 1024
QKV = 2048
T = 1024
DH = 128
NCORES = 8
NE = E // P       # 8
NC = QKV // P     # 16
EPS = float(np.finfo(np.float32).eps)
f32 = mybir.dt.float32
f32r = mybir.dt.float32r
bf16 = mybir.dt.bfloat16
AF = mybir.ActivationFunctionType
ALU = mybir.AluOpType
PAIRS = [[0, 1], [2, 3], [4, 5], [6, 7]]
BF = ml_dtypes.bfloat16


def _rsqrt(nc, pool, src_ps, scale, bias_ap, tag, ln_bufs=None):
    """rsqrt(src*scale + bias) = Exp(-0.5*Ln(.)). src_ps is PSUM (P, n)."""
    n = src_ps.shape[-1]
    tmp = pool.tile([P, n], f32, tag="lntmp", bufs=ln_bufs)
    nc.scalar.activation(tmp[:], src_ps[:], AF.Ln, scale=scale, bias=bias_ap)
    out = pool.tile([P, n], bf16, tag=tag)
    nc.scalar.activation(out[:], tmp[:], AF.Exp, scale=-0.5)
    return out


def _wtile8(nc, pool, wdram, m, tag="wk8"):
    """(P, 8, P) bf16 weight tile m from host-pretiled (M_total, K=8P) DRAM."""
    wt = pool.tile([P, NE, P], bf16, tag=tag)
    nc.sync.dma_start(wt[:], wdram.ap()[m * P:(m + 1) * P, :]
                      .rearrange("p (a n) -> p a n", n=P))
    return wt


def _build_aft_layer(tc, const, x_tiles, xp, wqkvT, wswiC, woutT,
                     ag_ins, ag_outs, x3_bf=None):
    """One AFT layer, fully SBUF-resident activations.

    x_tiles: list of 8 (P, TL) f32 SBUF tiles (residual stream).
    Returns the new list of 8 x tiles (allocated from xp).
    If x3_bf is given (layer 3), also writes the bf16 output to that DRAM AP.
    """
    nc = tc.nc
    ones_b = const["ones_b"]
    gate_col = const["gate"]

    with (
        tc.tile_pool(name="a_sc", bufs=2) as scp,
        tc.tile_pool(name="a_k", bufs=NC) as kp,
        tc.tile_pool(name="a_q", bufs=NC) as qp,
        tc.tile_pool(name="a_ww", bufs=NC) as wwp,
        tc.tile_pool(name="a_yf", bufs=NC) as yfp,
        tc.tile_pool(name="a_cc", bufs=8) as ccp,
        tc.tile_pool(name="a_xn", bufs=NE) as xnp,
    ):
        yf_t = [None] * NC
        w_t = [None] * NC
        wv_t = [None] * NC
        with (
            tc.tile_pool(name="a_w8", bufs=4) as wp,
            tc.tile_pool(name="a_ld", bufs=4) as sbp,
            tc.tile_pool(name="a_ps", bufs=4, space="PSUM") as ps,
            tc.tile_pool(name="a_ps2", bufs=1, space="PSUM") as ps2,
        ):
            # ---- rms(x) ----
            xsq = []
            for e in range(NE):
                t = sbp.tile([P, TL], bf16, tag="sq", bufs=NE)
                nc.gpsimd.tensor_tensor(t[:], x_tiles[e][:], x_tiles[e][:],
                                        ALU.mult)
                xsq.append(t)
            sumsq = ps2.tile([P, TL], f32, tag="xsumsq")
            for e in range(NE):
                nc.tensor.matmul(sumsq[:], ones_b[:], xsq[e][:],
                                 start=(e == 0), stop=(e == NE - 1))
            xscale = _rsqrt(nc, scp, sumsq, 1.0 / E, const["epsb"][:],
                            "scale")
            xn = []
            for e in range(NE):
                t = xnp.tile([P, TL], bf16, tag="xn")
                nc.vector.tensor_tensor(t[:], x_tiles[e][:], xscale[:],
                                        ALU.mult)
                xn.append(t)

            def qkv_mtile(m):
                wt = _wtile8(nc, wp, wqkvT, m)
                acc = ps.tile([P, TL], f32, tag="mm")
                for e in range(NE):
                    nc.tensor.matmul(acc[:], wt[:, e, :], xn[e][:],
                                     start=(e == 0), stop=(e == NE - 1))
                return acc

            # ---- k tiles (SBUF-resident bf16) ----
            k_sb = [None] * NC
            ksq = [None] * NC
            for c in range(NC):
                acc = qkv_mtile(16 + c)
                kt = kp.tile([P, TL], bf16, tag="k")
                nc.scalar.copy(kt[:], acc[:])
                k_sb[c] = kt
                sq = sbp.tile([P, TL], bf16, tag="sq", bufs=NE)
                nc.gpsimd.tensor_tensor(sq[:], kt[:], kt[:], ALU.mult)
                ksq[c] = sq
            ksumsq = ps2.tile([P, TL], f32, tag="ksumsq")
            for c in range(NC):
                nc.tensor.matmul(ksumsq[:], ones_b[:], ksq[c][:],
                                 start=(c == 0), stop=(c == NC - 1))
            kscale = _rsqrt(nc, scp, ksumsq, 1.0 / QKV, const["epsb"][:],
                            "scale")

            # ---- v matmuls + w/wv + carries (2 groups of 8) ----
            for g in range(2):
                for c in range(8 * g, 8 * g + 8):
                    kn = sbp.tile([P, TL], bf16, tag="kn", bufs=3)
                    nc.vector.tensor_tensor(kn[:], k_sb[c][:], kscale[:],
                                            ALU.mult)
                    w = wwp.tile([P, TL], bf16, tag="w")
                    cw_col = ccp.tile([P, 1], f32, tag="cwc")
                    nc.scalar.activation(w[:], kn[:], AF.Exp,
                                         accum_out=cw_col[:])
                    acc = qkv_mtile(32 + c)
                    wv = wwp.tile([P, TL], bf16, tag="wv")
                    cwv_col = ccp.tile([P, 1], f32, tag="cwvc")
                    nc.vector.scalar_tensor_tensor(
                        wv[:], acc[:], 0.0, w[:], ALU.bypass, ALU.mult,
                        accum_out=cwv_col[:])
                    j = c - 8 * g
                    nc.sync.dma_start(
                        ag_ins[g].opt()[:, j * P:(j + 1) * P]
                        .rearrange("o (p q) -> p (o q)", p=P),
                        cwv_col[:])
                    nc.sync.dma_start(
                        ag_ins[g].opt()[:, 1024 + j * P:1024 + (j + 1) * P]
                        .rearrange("o (p q) -> p (o q)", p=P),
                        cw_col[:])
                    w_t[c] = w
                    wv_t[c] = wv
                nc.gpsimd.collective_compute(
                    "AllGather", ALU.bypass, replica_groups=PAIRS,
                    ins=[ag_ins[g].opt()], outs=[ag_outs[g].opt()])

            # ---- q tiles (SBUF-resident bf16) ----
            q_sb = [None] * NC
            qsq = [None] * NC
            for c in range(NC):
                acc = qkv_mtile(c)
                qt = qp.tile([P, TL], bf16, tag="q")
                nc.scalar.copy(qt[:], acc[:])
                q_sb[c] = qt
                sq = sbp.tile([P, TL], bf16, tag="sq", bufs=NE)
                nc.gpsimd.tensor_tensor(sq[:], qt[:], qt[:], ALU.mult)
                qsq[c] = sq
            qsumsq = ps2.tile([P, TL], f32, tag="qsumsq")
            for c in range(NC):
                nc.tensor.matmul(qsumsq[:], ones_b[:], qsq[c][:],
                                 start=(c == 0), stop=(c == NC - 1))
            qscale = _rsqrt(nc, scp, qsumsq, 1.0 / QKV, const["epsb"][:],
                            "scale")

        # ---- phase B (scans etc.) interleaved with swiglu pass 1 ----
        with (
            tc.tile_pool(name="a_sw", bufs=4) as swp,
            tc.tile_pool(name="a_pb", bufs=3) as pbp,
            tc.tile_pool(name="a_u", bufs=NE) as up,
            tc.tile_pool(name="a_mt", bufs=NE) as mtp,
            tc.tile_pool(name="a_pss", bufs=8, space="PSUM") as pss,
        ):
            sacc = [None] * NE
            for g in range(2):
                cwv_raw = ccp.tile([P, 8], f32, tag="cwvr")
                nc.sync.dma_start(
                    cwv_raw[:], ag_outs[g].opt()[0:1, 0:1024]
                    .rearrange("o (c p) -> p (o c)", p=P))
                cw_raw = ccp.tile([P, 8], f32, tag="cwr")
                nc.sync.dma_start(
                    cw_raw[:], ag_outs[g].opt()[0:1, 1024:2048]
                    .rearrange("o (c p) -> p (o c)", p=P))
                cwv_g = ccp.tile([P, 8], f32, tag="cwvg")
                nc.vector.tensor_scalar(cwv_g[:], cwv_raw[:],
                                        gate_col[:], None, ALU.mult)
                # denominator carry gets the +1e-6 folded in
                cw_g = ccp.tile([P, 8], f32, tag="cwg")
                nc.vector.tensor_scalar(cw_g[:], cw_raw[:],
                                        gate_col[:], 1e-6,
                                        ALU.mult, ALU.add)
                for c in range(8 * g, 8 * g + 8):
                    j = c - 8 * g
                    sw = pbp.tile([P, TL], bf16, tag="sw")
                    nc.vector.tensor_tensor_scan(
                        sw[:], wv_t[c][:], wv_t[c][:], cwv_g[:, j:j + 1],
                        ALU.add, ALU.bypass)
                    sw2 = pbp.tile([P, TL], bf16, tag="sw2")
                    nc.vector.tensor_tensor_scan(
                        sw2[:], w_t[c][:], w_t[c][:], cw_g[:, j:j + 1],
                        ALU.add, ALU.bypass)
                    qn = pbp.tile([P, TL], bf16, tag="qn")
                    nc.gpsimd.tensor_tensor(qn[:], q_sb[c][:], qscale[:],
                                            ALU.mult)
                    et = pbp.tile([P, TL], bf16, tag="et")
                    nc.scalar.activation(et[:], qn[:], AF.Exp, scale=-1.0)
                    # dd = (et + 1) * sw2   (sw2 already carries the +1e-6)
                    dd = pbp.tile([P, TL], bf16, tag="dd")
                    nc.vector.scalar_tensor_tensor(
                        dd[:], et[:], 1.0, sw2[:], ALU.add, ALU.mult)
                    lnd = pbp.tile([P, TL], bf16, tag="lnd")
                    nc.scalar.activation(lnd[:], dd[:], AF.Ln)
                    rr = pbp.tile([P, TL], bf16, tag="rr")
                    nc.scalar.activation(rr[:], lnd[:], AF.Exp, scale=-1.0)
                    yf = yfp.tile([P, TL], bf16, tag="yf")
                    nc.vector.tensor_tensor(yf[:], sw[:], rr[:], ALU.mult)
                    yf_t[c] = yf
                    # swiglu pass 1 (u half, m=0..7), c-interleaved
                    wt = swp.tile([P, NE, P], bf16, tag="w1")
                    nc.sync.dma_start(
                        wt[:], wswiC.ap()[c * P:(c + 1) * P, 0:E]
                        .rearrange("p (a n) -> p a n", n=P))
                    for m in range(NE):
                        if c == 0:
                            sacc[m] = pss.tile([P, TL], f32, tag="sacc", name="sacc")
                        nc.tensor.matmul(sacc[m][:], wt[:, m, :], yf[:],
                                         start=(c == 0), stop=(c == NC - 1))

            # drain u, then swiglu pass 2 (g half, m=8..15), c-outer
            u_sb = [None] * NE
            for m in range(NE):
                ut = up.tile([P, TL], bf16, tag="u")
                nc.scalar.copy(ut[:], sacc[m][:])
                u_sb[m] = ut
            sacc2 = [None] * NE
            for c in range(NC):
                wt = swp.tile([P, NE, P], bf16, tag="w2")
                nc.sync.dma_start(
                    wt[:], wswiC.ap()[c * P:(c + 1) * P, E:2 * E]
                    .rearrange("p (a n) -> p a n", n=P))
                for m in range(NE):
                    if c == 0:
                        sacc2[m] = pss.tile([P, TL], f32, tag="sacc", name="sacc2")
                    nc.tensor.matmul(sacc2[m][:], wt[:, m, :], yf_t[c][:],
                                     start=(c == 0), stop=(c == NC - 1))
            # silu: m = u * g / (1 + exp(-g))
            m_t = [None] * NE
            for m in range(NE):
                eg = pbp.tile([P, TL], bf16, tag="eg")
                nc.scalar.activation(eg[:], sacc2[m][:], AF.Exp, scale=-1.0)
                lnd = pbp.tile([P, TL], bf16, tag="lnd")
                nc.scalar.activation(lnd[:], eg[:], AF.Ln,
                                     bias=const["oneb"][:])
                rr = pbp.tile([P, TL], bf16, tag="rr")
                nc.scalar.activation(rr[:], lnd[:], AF.Exp, scale=-1.0)
                pug = pbp.tile([P, TL], bf16, tag="pug")
                nc.vector.tensor_tensor(pug[:], u_sb[m][:], sacc2[m][:],
                                        ALU.mult)
                mt = mtp.tile([P, TL], bf16, tag="mt")
                nc.gpsimd.tensor_tensor(mt[:], pug[:], rr[:], ALU.mult)
                m_t[m] = mt

            # ---- out-proj + residual (SBUF resident) ----
            new_x = []
            with tc.tile_pool(name="a_w8b", bufs=3) as wpb:
                for mo in range(NE):
                    wt = _wtile8(nc, wpb, woutT, mo)
                    acc = pss.tile([P, TL], f32, tag="sacc")
                    for c in range(NE):
                        nc.tensor.matmul(acc[:], wt[:, c, :], m_t[c][:],
                                         start=(c == 0), stop=(c == NE - 1))
                    xo = xp.tile([P, TL], f32, tag="x", bufs=10)
                    nc.vector.tensor_tensor(xo[:], acc[:], x_tiles[mo][:],
                                            ALU.add)
                    new_x.append(xo)
                    if x3_bf is not None:
                        xob = pbp.tile([P, TL], bf16, tag="xob")
                        nc.scalar.copy(xob[:], xo[:])
                        nc.sync.dma_start(
                            x3_bf[mo * P:(mo + 1) * P, :], xob[:])
    return new_x


def _build_tea(tc, const, x_tiles, wqk4c, wv4c, wswiT4c, woutT4,
               agx_out_h, rs_in_h, rs_out_h, outT):
    nc = tc.nc
    ones_r = const["ones_r"]
    cc_t, ss_t, cm_t = const["cc"], const["ss"], const["cmask"]
    HL = 8

    with (
        tc.tile_pool(name="t_yt", bufs=2 * HL) as ytp,
        tc.tile_pool(name="t_sc", bufs=2) as scp,
        tc.tile_pool(name="t_ps", bufs=2, space="PSUM") as ps,
        tc.tile_pool(name="t_ps2", bufs=2, space="PSUM") as ps2,
        tc.tile_pool(name="t_xn", bufs=2 * NE) as xnp,
        tc.tile_pool(name="t_v", bufs=16) as vp,
    ):
        with tc.tile_pool(name="t_t", bufs=3) as sbp:
            # ---- rms(x3) (x3 arrives bf16 via the pair AllGather) ----
            xn = [[None] * NE for _ in range(2)]
            for tch in range(2):
                def _x3_ap(tch, e):
                    half, er = e // 4, e % 4
                    return agx_out_h[half].opt()[
                        tch * (E // 2) + er * P:tch * (E // 2) + (er + 1) * P, :]

                xt3s = []
                for e in range(NE):
                    xt3 = sbp.tile([P, TL], bf16, tag="xt3", bufs=NE)
                    nc.sync.dma_start(xt3[:], _x3_ap(tch, e))
                    xt3s.append(xt3)
                sumsq = ps2.tile([P, TL], f32, tag="sumsq")
                for e in range(NE):
                    xsq = sbp.tile([P, TL], bf16, tag="sq")
                    nc.gpsimd.tensor_tensor(xsq[:], xt3s[e][:], xt3s[e][:],
                                            ALU.mult)
                    nc.tensor.matmul(sumsq[:], const["ones_b"][:], xsq[:],
                                     start=(e == 0), stop=(e == NE - 1))
                xscale = _rsqrt(nc, scp, sumsq, 1.0 / E, const["epsb"][:],
                                "xscale", ln_bufs=2)
                for e in range(NE):
                    t = xnp.tile([P, TL], bf16, tag="xn")
                    nc.vector.tensor_tensor(t[:], xt3s[e][:], xscale[:],
                                            ALU.mult)
                    xn[tch][e] = t

            # ---- V (token-major) ----
            V = [[None] * 2 for _ in range(8)]
            with tc.tile_pool(name="t_vw", bufs=2) as vwp:
                for vb in range(2):
                    vw = vwp.tile([P, NE, TL], bf16, tag="vw")
                    nc.sync.dma_start(
                        vw[:],
                        wv4c.ap()[vb * P:(vb + 1) * P, :]
                        .rearrange("p (a n) -> p a n", n=TL))
                    for ttile in range(8):
                        tch, toff = ttile // 4, (ttile % 4) * P
                        acc = ps.tile([P, TL], f32, tag="mm")
                        for e in range(NE):
                            nc.tensor.matmul(
                                acc[:], xn[tch][e][:, toff:toff + P],
                                vw[:, e, :],
                                start=(e == 0), stop=(e == NE - 1))
                        vt = vp.tile([P, TL], bf16, tag="V")
                        nc.scalar.copy(vt[:], acc[:])
                        V[ttile][vb] = vt

        # ---- per-head rope/rms + attention ----
        yT = [[None] * 2 for _ in range(HL)]
        with (
            tc.tile_pool(name="t_qk", bufs=6) as qkp,
            tc.tile_pool(name="t_es", bufs=8) as esp,
            tc.tile_pool(name="t_w8", bufs=3) as wp,
            tc.tile_pool(name="t_at", bufs=2) as sba,
            tc.tile_pool(name="t_psa", bufs=2, space="PSUM") as psa,
            tc.tile_pool(name="t_psd", bufs=1, space="PSUM") as psd,
        ):
            sel4 = const["sel4"]
            for h in range(HL):
                qn_h = [None] * 2
                kn_h = [None] * 2
                sites = []
                coll = scp.tile([4, TL], f32, tag="coll", bufs=2)
                # pass 1: matmuls, (1xTL) sumsq rows into the collector, rope
                for wi, (which, mti, out_list) in enumerate(
                        (("q", h, qn_h), ("k", NE + h, kn_h))):
                    wt = _wtile8(nc, wp, wqk4c, mti)
                    for tch in range(2):
                        acc = ps.tile([P, TL], f32, tag="mm")
                        for e in range(NE):
                            nc.tensor.matmul(acc[:], wt[:, e, :],
                                             xn[tch][e][:],
                                             start=(e == 0),
                                             stop=(e == NE - 1))
                        zsq = sba.tile([P, TL], f32r, tag="sq")
                        nc.scalar.activation(zsq[:], acc[:], AF.Square)
                        sq_ps = ps2.tile([1, TL], f32, tag="sumsq")
                        nc.tensor.matmul(sq_ps[:], ones_r[:, 0:1], zsq[:],
                                         start=True, stop=True)
                        r = 2 * wi + tch
                        srow = scp.tile([1, TL], f32, tag="srow", bufs=3)
                        nc.scalar.copy(srow[:], sq_ps[:])
                        nc.sync.dma_start(coll[r:r + 1, :], srow[:])
                        tsl = slice(tch * TL, (tch + 1) * TL)
                        tmp1 = sba.tile([P, TL], f32, tag="tmp1")
                        nc.vector.tensor_tensor(tmp1[:], acc[:],
                                                cc_t[:, tsl], ALU.mult)
                        cross = sba.tile([P, TL], f32, tag="cross")
                        nc.vector.tensor_tensor(cross[:64, :], acc[64:, :],
                                                ss_t[:64, tsl], ALU.mult)
                        nc.vector.tensor_tensor(cross[64:, :], acc[:64, :],
                                                ss_t[64:, tsl], ALU.mult)
                        zrope = sba.tile([P, TL], f32, tag="zrope",
                                         bufs=3)
                        nc.gpsimd.tensor_tensor(zrope[:], tmp1[:], cross[:],
                                                ALU.add)
                        sites.append((r, zrope, out_list, tch))
                # one Ln + one Exp for all 4 sites of this head.
                lnc = scp.tile([4, TL], f32, tag="lnc", bufs=2)
                nc.scalar.activation(lnc[:], coll[:], AF.Ln,
                                     bias=const["epsbdh"][0:4, :])
                esc = scp.tile([4, TL], f32r, tag="esc", bufs=2)
                nc.scalar.activation(esc[:], lnc[:], AF.Exp, scale=-0.5,
                                     bias=const["klnb"][:])
                for r, zrope, out_list, tch in sites:
                    sc_ps = ps.tile([P, TL], f32, tag="mm")
                    nc.tensor.matmul(sc_ps[:], sel4[:, r * P:(r + 1) * P],
                                     esc[:], start=True, stop=True)
                    zn = qkp.tile([P, TL], f32r, tag="zn")
                    nc.vector.tensor_tensor(zn[:], zrope[:], sc_ps[:],
                                            ALU.mult)
                    out_list[tch] = zn

                for qc in range(2):
                    denom = psd.tile([P, TL], f32, tag="denom")
                    ytil = psd.tile([P, TL], f32, tag="ytil")
                    nkt = 4 * (qc + 1)
                    for kt in range(nkt):
                        tch_k, koff = kt // 4, (kt % 4) * P
                        sT = psa.tile([P, TL], f32, tag="sT")
                        nc.tensor.matmul(sT[:],
                                         kn_h[tch_k][:, koff:koff + P],
                                         qn_h[qc][:], start=True, stop=True)
                        es = esp.tile([P, TL], bf16, tag="es")
                        j = kt - 4 * qc
                        if j >= 0:
                            sm = sba.tile([P, TL], f32, tag="sm")
                            nc.vector.tensor_tensor(
                                sm[:], sT[:], cm_t[:, j * TL:(j + 1) * TL],
                                ALU.add)
                            nc.scalar.activation(es[:], sm[:], AF.Exp)
                        else:
                            nc.scalar.activation(es[:], sT[:], AF.Exp)
                        nc.tensor.matmul(denom[:], const["ones_b"][:], es[:],
                                         start=(kt == 0),
                                         stop=(kt == nkt - 1))
                        nc.tensor.matmul(
                            ytil[:],
                            V[kt][h // 4][:, (h % 4) * P:(h % 4 + 1) * P],
                            es[:], start=(kt == 0), stop=(kt == nkt - 1))
                    lnr = sba.tile([P, TL], f32, tag="lnr")
                    nc.scalar.activation(lnr[:], denom[:], AF.Ln)
                    rr = sba.tile([P, TL], f32, tag="arr")
                    nc.scalar.activation(rr[:], lnr[:], AF.Exp, scale=-1.0)
                    yt = ytp.tile([P, TL], bf16, tag="yT")
                    nc.vector.tensor_tensor(yt[:], ytil[:], rr[:], ALU.mult)
                    yT[h][qc] = yt

        # ---- partial swiglu (all T tokens, my y channels) ----
        with (
            tc.tile_pool(name="t_w8s", bufs=3) as wps,
            tc.tile_pool(name="t_pug", bufs=4) as pugp,
        ):
            # rs half h holds m-tiles [4h..4h+4) (u) and [8+4h..8+4h+4) (g)
            def _rs_slot(m):
                h = (m % NE) // 4
                loc = (m % 4) + 4 * (m // NE)   # 0..7 within half
                return h, loc

            for h in range(2):
                for m in list(range(4 * h, 4 * h + 4)) + \
                        list(range(NE + 4 * h, NE + 4 * h + 4)):
                    wt = _wtile8(nc, wps, wswiT4c, m)
                    _, loc = _rs_slot(m)
                    for tch in range(2):
                        acc = ps.tile([P, TL], f32, tag="mm")
                        for kk in range(HL):
                            nc.tensor.matmul(acc[:], wt[:, kk, :],
                                             yT[kk][tch][:],
                                             start=(kk == 0),
                                             stop=(kk == HL - 1))
                        pug = pugp.tile([P, TL], bf16, tag="pug")
                        nc.scalar.copy(pug[:], acc[:])
                        nc.sync.dma_start(
                            rs_in_h[h].opt()[tch * E + loc * P:
                                             tch * E + (loc + 1) * P, :],
                            pug[:])
                nc.gpsimd.collective_compute(
                    "ReduceScatter", ALU.add, replica_groups=PAIRS,
                    ins=[rs_in_h[h].opt()], outs=[rs_out_h[h].opt()])

        # ---- silu + out-proj + residual ----
        with (
            tc.tile_pool(name="t_mt", bufs=NE) as mtp,
            tc.tile_pool(name="t_w8o", bufs=3) as wpo,
            tc.tile_pool(name="t_t4", bufs=2) as sb4,
        ):
            m_t = [None] * NE
            for c in range(NE):
                h, cr = c // 4, c % 4
                ut = sb4.tile([P, TL], bf16, tag="u4")
                nc.sync.dma_start(
                    ut[:], rs_out_h[h].opt()[cr * P:(cr + 1) * P, :])
                gt = sb4.tile([P, TL], bf16, tag="g4")
                nc.sync.dma_start(
                    gt[:],
                    rs_out_h[h].opt()[TL + cr * P:TL + (cr + 1) * P, :])
                eg = sb4.tile([P, TL], bf16, tag="eg4")
                nc.scalar.activation(eg[:], gt[:], AF.Exp, scale=-1.0)
                lnd = sb4.tile([P, TL], bf16, tag="lnd4")
                nc.scalar.activation(lnd[:], eg[:], AF.Ln,
                                     bias=const["oneb"][:])
                rr = sb4.tile([P, TL], bf16, tag="rr4")
                nc.scalar.activation(rr[:], lnd[:], AF.Exp, scale=-1.0)
                pug = sb4.tile([P, TL], bf16, tag="pug4")
                nc.gpsimd.tensor_tensor(pug[:], ut[:], gt[:], ALU.mult)
                mt = mtp.tile([P, TL], bf16, tag="mt4")
                nc.vector.tensor_tensor(mt[:], pug[:], rr[:], ALU.mult)
                m_t[c] = mt
            for mo in range(NE):
                wt = _wtile8(nc, wpo, woutT4, mo)
                acc = ps.tile([P, TL], f32, tag="mm")
                for c in range(NE):
                    nc.tensor.matmul(acc[:], wt[:, c, :], m_t[c][:],
                                     start=(c == 0), stop=(c == NE - 1))
                xo = sb4.tile([P, TL], f32, tag="xo4")
                nc.vector.tensor_tensor(xo[:], acc[:], x_tiles[mo][:],
                                        ALU.add)
                nc.sync.dma_start(outT.ap()[mo * P:(mo + 1) * P, :], xo[:])


class _Bacc(bacc.Bacc):
    """Bacc with the combined ln+exp activation table given priority.

    The act-table insertion pass assigns each activation the first table
    in the list that contains its function; the default act_info order
    makes Exp resolve to `exp_and_others` and Ln to `natural_log`, so a
    kernel that alternates Exp/Ln (reciprocals, rsqrt) reloads the table
    on nearly every call (~1.3us each). Putting
    `natural_log_exp_and_others` first lets Exp/Ln/Square/Copy all share
    one resident table.
    """

    def insert_act_table_loads(self):
        import bass_rust as _bass_rust
        from concourse.hw_specs import get_activation_tables
        has_activation = any(
            isinstance(i, mybir.InstActivation)
            for b in self.main_func.blocks
            for i in b.instructions
        )
        if not has_activation:
            return
        steer = {AF.Exp, AF.Ln, AF.Square, AF.Copy}
        tables = [
            (nm, set(fns) if nm == 'natural_log_exp_and_others'
             else set(fns) - steer)
            for nm, fns in get_activation_tables(self.m.arch).items()
        ]
        _bass_rust.insert_act_table_loads(self, tables)


def build_program():
    nc = _Bacc("TRN2", target_bir_lowering=False, debug=False,
               num_devices=NCORES)

    din = {}

    def inp(name, shape, dt):
        din[name] = nc.dram_tensor(name, list(shape), dt,
                                   kind="ExternalInput")
        return din[name]

    inp("xT0", (E, TL), f32)
    for l in (1, 2, 3):
        inp(f"wqkvT{l}", (3 * QKV, E), bf16)       # tile layout [m*P, K]
        inp(f"wswiC{l}", (QKV, 2 * E), bf16)       # plain W.T (K rows, M cols)
        inp(f"woutT{l}", (E, E), bf16)
    inp("wqk4c", (QKV, E), bf16)                   # [q_h0..q_h7, k_h0..k_h7]
    inp("wv4c", (2 * P, NE * TL), bf16)            # [vb, p, e, n]
    inp("wswiT4c", (QKV, E), bf16)
    inp("woutT4", (E, E), bf16)
    inp("cc", (P, T), f32)
    inp("ss", (P, T), f32)
    inp("cmask", (P, 4 * TL), f32)
    inp("gate", (P, 1), f32)
    inp("ones_r", (P, P), f32r)
    inp("ones_b", (P, P), bf16)
    inp("sel4", (4, 4 * P), f32r)
    inp("klnb", (4, 1), f32)
    outT = nc.dram_tensor("outT", [E, TL], f32, kind="ExternalOutput")

    with tile.TileContext(nc) as tc:
        with (
            tc.tile_pool(name="const", bufs=1) as constp,
            tc.tile_pool(name="xres", bufs=10) as xp,
            tc.tile_pool(name="dram", bufs=1, space="DRAM") as dram,
        ):
            const = {}
            epsb = constp.tile([P, 1], f32, tag="epsb")
            nc.any.memset(epsb[:], EPS)
            const["epsb"] = epsb
            epsbdh = constp.tile([P, 1], f32, tag="epsbdh")
            nc.any.memset(epsbdh[:], DH * EPS)
            const["epsbdh"] = epsbdh
            oneb = constp.tile([P, 1], f32, tag="oneb")
            nc.any.memset(oneb[:], 1.0)
            const["oneb"] = oneb
            for nm, dt in (("cc", f32), ("ss", f32), ("cmask", f32),
                           ("gate", f32), ("ones_r", f32r),
                           ("ones_b", bf16), ("sel4", f32r), ("klnb", f32)):
                t = constp.tile(list(din[nm].shape), dt, tag=nm)
                nc.sync.dma_start(t[:], din[nm].ap())
                const[nm] = t

            # load residual stream into SBUF once
            x_tiles = []
            for e in range(NE):
                xt = xp.tile([P, TL], f32, tag="x", bufs=10)
                nc.sync.dma_start(xt[:], din["xT0"].ap()[e * P:(e + 1) * P, :])
                x_tiles.append(xt)

            agx_in = dram.tile([E, TL], bf16, tag="agx", name="agx")
            for l in (1, 2, 3):
                ag_ins = [dram.tile([1, 2048], f32, tag=f"agi{l}_{g}",
                                    name=f"agi{l}_{g}") for g in range(2)]
                ag_outs = [dram.tile([2, 2048], f32, tag=f"ago{l}_{g}",
                                     name=f"ago{l}_{g}") for g in range(2)]
                x_tiles = _build_aft_layer(
                    tc, const, x_tiles, xp,
                    din[f"wqkvT{l}"], din[f"wswiC{l}"], din[f"woutT{l}"],
                    ag_ins, ag_outs,
                    x3_bf=(agx_in.opt() if l == 3 else None))

            agx_out_h = [dram.tile([E, TL], bf16, tag=f"agxo{h}",
                                   name=f"agxo{h}") for h in range(2)]
            for half in range(2):
                nc.gpsimd.collective_compute(
                    "AllGather", ALU.bypass, replica_groups=PAIRS,
                    ins=[agx_in.opt()[half * (E // 2):(half + 1) * (E // 2), :]],
                    outs=[agx_out_h[half].opt()])
            rs_in_h = [dram.tile([QKV, TL], bf16, tag=f"rsi{h}",
                                 name=f"rsi{h}") for h in range(2)]
            rs_out_h = [dram.tile([E, TL], bf16, tag=f"rso{h}",
                                  name=f"rso{h}") for h in range(2)]
            _build_tea(tc, const, x_tiles, din["wqk4c"], din["wv4c"],
                       din["wswiT4c"], din["woutT4"], agx_out_h,
                       rs_in_h, rs_out_h, outT)

    nc.compile()
    return nc


# --------------------------------------------------------------------------
# host-side sharding
# --------------------------------------------------------------------------

def _host_inputs(inputs):
    f = np.float32
    cos = np.ascontiguousarray(np.asarray(inputs['cos'], f)[:, 0, :].T)
    sin = np.ascontiguousarray(np.asarray(inputs['sin'], f)[:, 0, :].T)
    cc = np.concatenate([cos, cos], 0)
    ss = np.concatenate([sin, -sin], 0)
    cm = np.zeros((4, P, TL), f)
    kk = np.arange(P)[:, None]
    qq = np.arange(TL)[None, :]
    for j in range(4):
        cm[j] = np.where(P * j + kk <= qq, 0.0, -1e30)
    cmask = np.ascontiguousarray(cm.transpose(1, 0, 2).reshape(P, 4 * TL))
    ones_r = np.ones((P, P), f)
    ones_b = np.ones((P, P), BF)

    def tl(wT):
        # (K, M) -> tile layout (M, K): row-block m = [p, e, n] contiguous
        K, M = wT.shape
        return np.ascontiguousarray(
            wT.reshape(K // P, P, M // P, P).transpose(2, 1, 0, 3)
            .reshape(M, K))

    sel4 = np.zeros((4, 4 * P), f)
    for i in range(4):
        sel4[i, i * P:(i + 1) * P] = 1.0
    klnb = np.array([[0.0], [0.0], [0.5 * np.log(DH)], [0.5 * np.log(DH)]], f)
    shared = {'cc': cc, 'ss': ss, 'cmask': cmask, 'ones_r': ones_r,
              'ones_b': ones_b, 'sel4': sel4, 'klnb': klnb}
    for l in (1, 2, 3):
        shared[f'wqkvT{l}'] = tl(np.asarray(inputs[f'w_qkv{l}'], f).T).astype(BF)
        shared[f'wswiC{l}'] = np.ascontiguousarray(
            np.asarray(inputs[f'w_swiglu{l}'], f).T).astype(BF)
        shared[f'woutT{l}'] = tl(np.asarray(inputs[f'w_out{l}'], f).T).astype(BF)
    shared['woutT4'] = tl(np.asarray(inputs['w_out4'], f).T).astype(BF)

    wq4 = np.asarray(inputs['w_qkv4'], f).T       # (E, 6144): per-head blocks
    wswi4 = np.asarray(inputs['w_swiglu4'], f).T  # (QKV, 2E)
    by_par = {}
    for par in range(2):
        hs = par * 8
        qk_cols = []
        for part in range(2):   # q then k blocks
            for h in range(hs, hs + 8):
                qk_cols.append(wq4[:, h * 3 * DH + part * DH:
                                   h * 3 * DH + (part + 1) * DH])
        v_cols = [wq4[:, h * 3 * DH + 2 * DH: h * 3 * DH + 3 * DH]
                  for h in range(hs, hs + 8)]
        kv = np.concatenate(v_cols, 1)             # (E, 1024)
        # wv4c layout [vb, p, e, n]: element = kv[128e + p, vb*512 + n]
        wv4c = np.ascontiguousarray(
            kv.reshape(NE, P, 2, TL).transpose(2, 1, 0, 3)
            .reshape(2 * P, NE * TL))
        by_par[par] = {
            'wqk4c': tl(np.concatenate(qk_cols, 1)).astype(BF),
            'wv4c': wv4c.astype(BF),
            'wswiT4c': tl(np.ascontiguousarray(
                wswi4[hs * DH:(hs + 8) * DH, :])).astype(BF),
            'gate': np.full((P, 1), float(par), f),
        }

    x = np.asarray(inputs['x'], f)
    in_maps = []
    for c in range(NCORES):
        b, par = c // 2, c % 2
        m = dict(shared)
        m.update(by_par[par])
        m['xT0'] = np.ascontiguousarray(x[b, par * TL:(par + 1) * TL, :].T)
        in_maps.append(m)
    return in_maps


_cached = {}


def kernel(**inputs):
    if 'nc' not in _cached:
        _cached['nc'] = build_program()
    nc = _cached['nc']
    in_maps = _host_inputs(inputs)
    trace = bool(int(os.environ.get('BASS_KERNEL_TRACE', '0')))
    res = run_bass_kernel_spmd(nc, in_maps, core_ids=list(range(NCORES)),
                               trace=trace)
    _cached['last_results'] = res
    out = np.zeros((4, T, E), np.float32)
    for c in range(NCORES):
        b, par = c // 2, c % 2
        out[b, par * TL:(par + 1) * TL, :] = res.results[c]['outT'].T
    return out


# revision 39
# speedup vs baseline: 2.1110x; 1.0425x over previous
"""TRN2 Bass kernel for nn_Block_82325933129820.

3x AFT blocks + 1 transformer (TEA) block, B=4 T=1024 E=1024 QKV=2048 H=16.

Sharding: 8 cores = 4 batch-pairs. Within a pair (even core, odd core):
  - AFT layers: token-split (even: tokens 0-511, odd: 512-1023), feature-major
    activations (channels on partitions, tokens on free dim). The cumsum runs
    as per-chunk tensor_tensor_scan along the free dim; cross-core carries
    travel via pair AllGathers and enter as the scan's `initial` value, gated
    to zero on even cores (with the denominator's +1e-6 folded in).
  - TEA: head-split (even: heads 0-7, odd: 8-15) over the full 1024 tokens.
    x3 is pair-AllGathered in bf16; attention is computed in S^T layout; the
    swiglu partial contraction is pair-ReduceScattered in bf16.

Precision/perf strategy: all GEMM weights are bf16 (half the HBM traffic);
PSUM accumulation is fp32. AFT intermediate activations (q/k/w/wv/yf) are
bf16 which enables DVE 2x modes; the cumsum scan recurrence is fp32
internally regardless. TEA attention internals stay fp32 (f32r matmuls at
full PE rate for N>=512). All reciprocals run on the scalar engine as
Exp(-Ln(x)); rsqrt(x) = Exp(-0.5*Ln(x)); sigmoid/silu via Exp with the
reciprocal folded into existing products; "+1"/"+eps" constants folded into
activation bias / scan initials. The AFT swiglu's first 8 output tiles
accumulate c-interleaved with the cumsum pipeline so the tensor engine
stays busy through the vector-heavy phase.
"""
import os
import sys
import numpy as np
import ml_dtypes

for _p in ('/opt/trn_rl_repo',):
    if _p not in sys.path:
        sys.path.insert(0, _p)

import concourse.bass as bass
import concourse.mybir as mybir
import concourse.tile as tile
from concourse import bacc
from concourse.bass_utils import run_bass_kernel_spmd

P = 128
TL = 512          # AFT tokens per core
E = 1024
QKV = 2048
T = 1024
DH = 128
NCORES = 8
NE = E // P       # 8
NC = QKV // P     # 16
EPS = float(np.finfo(np.float32).eps)
f32 = mybir.dt.float32
f32r = mybir.dt.float32r
bf16 = mybir.dt.bfloat16
AF = mybir.ActivationFunctionType
ALU = mybir.AluOpType
PAIRS = [[0, 1], [2, 3], [4, 5], [6, 7]]
BF = ml_dtypes.bfloat16


def _rsqrt(nc, pool, src_ps, scale, bias_ap, tag, ln_bufs=None):
    """rsqrt(src*scale + bias) = Exp(-0.5*Ln(.)). src_ps is PSUM (P, n)."""
    n = src_ps.shape[-1]
    tmp = pool.tile([P, n], f32, tag="lntmp", bufs=ln_bufs)
    nc.scalar.activation(tmp[:], src_ps[:], AF.Ln, scale=scale, bias=bias_ap)
    out = pool.tile([P, n], bf16, tag=tag)
    nc.scalar.activation(out[:], tmp[:], AF.Exp, scale=-0.5)
    return out


def _wgroup(nc, pool, wdram, m0, G, K=E, tag="wkg", bufs=None, name="wt"):
    """(P, G, K//P, P) bf16 weight group from host-packed (P, Mtiles*K) DRAM.

    Row p of the DRAM tensor holds tile-m-major data, so a G-tile load is
    one G*K*2-byte contiguous chunk per partition (fat DMA descriptors).
    """
    wt = pool.tile([P, G, K // P, P], bf16, tag=tag, bufs=bufs, name=name)
    # two partition-half dma_starts: same fat per-row descriptors, but two
    # hardware queues stream concurrently (per-stream DMA bw is the limiter)
    for lo, hi in ((0, P // 2), (P // 2, P)):
        nc.sync.dma_start(wt[lo:hi], wdram.ap()[lo:hi, m0 * K:(m0 + G) * K]
                          .rearrange("p (b a n) -> p b a n", b=G, n=P))
    return wt


def _build_aft_layer(tc, const, x_tiles, xp, wqkvT, wswiU, wswiG,
                     woutT, ag_ins, ag_outs, x3_bf=None):
    """One AFT layer, fully SBUF-resident activations.

    x_tiles: list of 8 (P, TL) f32 SBUF tiles (residual stream).
    Returns the new list of 8 x tiles (allocated from xp).
    If x3_bf is given (layer 3), also writes the bf16 output to that DRAM AP.
    """
    nc = tc.nc
    ones_b = const["ones_b"]
    gate_col = const["gate"]

    with (
        tc.tile_pool(name="a_sc", bufs=2) as scp,
        tc.tile_pool(name="a_k", bufs=NC) as kp,
        tc.tile_pool(name="a_q", bufs=NC) as qp,
        tc.tile_pool(name="a_ww", bufs=NC) as wwp,
        tc.tile_pool(name="a_yf", bufs=NC) as yfp,
        tc.tile_pool(name="a_cc", bufs=8) as ccp,
        tc.tile_pool(name="a_xn", bufs=NE) as xnp,
    ):
        yf_t = [None] * NC
        w_t = [None] * NC
        wv_t = [None] * NC
        with (
            tc.tile_pool(name="a_w8", bufs=4) as wp,
            tc.tile_pool(name="a_ld", bufs=4) as sbp,
            tc.tile_pool(name="a_ps", bufs=4, space="PSUM") as ps,
            tc.tile_pool(name="a_ps2", bufs=1, space="PSUM") as ps2,
        ):
            # ---- rms(x) ----
            xsq = []
            for e in range(NE):
                t = sbp.tile([P, TL], bf16, tag="sq", bufs=NE)
                nc.gpsimd.tensor_tensor(t[:], x_tiles[e][:], x_tiles[e][:],
                                        ALU.mult)
                xsq.append(t)
            sumsq = ps2.tile([P, TL], f32, tag="xsumsq")
            for e in range(NE):
                nc.tensor.matmul(sumsq[:], ones_b[:], xsq[e][:],
                                 start=(e == 0), stop=(e == NE - 1))
            xscale = _rsqrt(nc, scp, sumsq, 1.0 / E, const["epsb"][:],
                            "scale")
            xn = []
            for e in range(NE):
                t = xnp.tile([P, TL], bf16, tag="xn")
                nc.vector.tensor_tensor(t[:], x_tiles[e][:], xscale[:],
                                        ALU.mult)
                xn.append(t)

            def qkv_group(mt0, gblk):
                """Load G=4 qkv weight tiles, return the group tile."""
                return _wgroup(nc, wp, wqkvT, mt0 + 4 * gblk, 4, tag="wk4")

            def qkv_acc(wt, b):
                acc = ps.tile([P, TL], f32, tag="mm", name="acc")
                for e in range(NE):
                    nc.tensor.matmul(acc[:], wt[:, b, e, :], xn[e][:],
                                     start=(e == 0), stop=(e == NE - 1))
                return acc

            # ---- k tiles (SBUF-resident bf16); k weight tiles are m 16..31
            k_sb = [None] * NC
            ksq = [None] * NC
            for gblk in range(4):
                wt = qkv_group(16, gblk)
                for b in range(4):
                    c = 4 * gblk + b
                    acc = qkv_acc(wt, b)
                    kt = kp.tile([P, TL], bf16, tag="k")
                    nc.scalar.copy(kt[:], acc[:])
                    k_sb[c] = kt
                    sq = sbp.tile([P, TL], bf16, tag="sq", bufs=NE)
                    nc.gpsimd.tensor_tensor(sq[:], kt[:], kt[:], ALU.mult)
                    ksq[c] = sq
            ksumsq = ps2.tile([P, TL], f32, tag="ksumsq")
            for c in range(NC):
                nc.tensor.matmul(ksumsq[:], ones_b[:], ksq[c][:],
                                 start=(c == 0), stop=(c == NC - 1))
            kscale = _rsqrt(nc, scp, ksumsq, 1.0 / QKV, const["epsb"][:],
                            "scale")

            # ---- v matmuls + w/wv + carries (2 groups of 8); v is m 32..47
            for g in range(2):
                for gblk in range(2 * g, 2 * g + 2):
                    wt = qkv_group(32, gblk)
                    for b in range(4):
                        c = 4 * gblk + b
                        kn = sbp.tile([P, TL], bf16, tag="kn", bufs=3)
                        nc.vector.tensor_tensor(kn[:], k_sb[c][:],
                                                kscale[:], ALU.mult)
                        w = wwp.tile([P, TL], bf16, tag="w")
                        cw_col = ccp.tile([P, 1], f32, tag="cwc")
                        nc.scalar.activation(w[:], kn[:], AF.Exp,
                                             accum_out=cw_col[:])
                        acc = qkv_acc(wt, b)
                        wv = wwp.tile([P, TL], bf16, tag="wv")
                        cwv_col = ccp.tile([P, 1], f32, tag="cwvc")
                        nc.vector.scalar_tensor_tensor(
                            wv[:], acc[:], 0.0, w[:], ALU.bypass, ALU.mult,
                            accum_out=cwv_col[:])
                        j = c - 8 * g
                        nc.sync.dma_start(
                            ag_ins[g].opt()[:, j * P:(j + 1) * P]
                            .rearrange("o (p q) -> p (o q)", p=P),
                            cwv_col[:])
                        nc.sync.dma_start(
                            ag_ins[g].opt()[:, 1024 + j * P:1024 + (j + 1) * P]
                            .rearrange("o (p q) -> p (o q)", p=P),
                            cw_col[:])
                        w_t[c] = w
                        wv_t[c] = wv
                nc.gpsimd.collective_compute(
                    "AllGather", ALU.bypass, replica_groups=PAIRS,
                    ins=[ag_ins[g].opt()], outs=[ag_outs[g].opt()])

            # ---- q tiles (SBUF-resident bf16); q is m 0..15 ----
            q_sb = [None] * NC
            qsq = [None] * NC
            for gblk in range(4):
                wt = qkv_group(0, gblk)
                for b in range(4):
                    c = 4 * gblk + b
                    acc = qkv_acc(wt, b)
                    qt = qp.tile([P, TL], bf16, tag="q")
                    nc.scalar.copy(qt[:], acc[:])
                    q_sb[c] = qt
                    sq = sbp.tile([P, TL], bf16, tag="sq", bufs=NE)
                    nc.gpsimd.tensor_tensor(sq[:], qt[:], qt[:], ALU.mult)
                    qsq[c] = sq
            qsumsq = ps2.tile([P, TL], f32, tag="qsumsq")
            for c in range(NC):
                nc.tensor.matmul(qsumsq[:], ones_b[:], qsq[c][:],
                                 start=(c == 0), stop=(c == NC - 1))
            qscale = _rsqrt(nc, scp, qsumsq, 1.0 / QKV, const["epsb"][:],
                            "scale")

        # ---- phase B (scans etc.) interleaved with swiglu pass 1 ----
        with (
            tc.tile_pool(name="a_sw", bufs=2) as swp,
            tc.tile_pool(name="a_pb", bufs=2) as pbp,
            tc.tile_pool(name="a_u", bufs=NE) as up,
            tc.tile_pool(name="a_mt", bufs=NE) as mtp,
            tc.tile_pool(name="a_pss", bufs=8, space="PSUM") as pss,
        ):
            sacc = [None] * NE
            for g in range(2):
                cwv_raw = ccp.tile([P, 8], f32, tag="cwvr")
                nc.sync.dma_start(
                    cwv_raw[:], ag_outs[g].opt()[0:1, 0:1024]
                    .rearrange("o (c p) -> p (o c)", p=P))
                cw_raw = ccp.tile([P, 8], f32, tag="cwr")
                nc.sync.dma_start(
                    cw_raw[:], ag_outs[g].opt()[0:1, 1024:2048]
                    .rearrange("o (c p) -> p (o c)", p=P))
                cwv_g = ccp.tile([P, 8], f32, tag="cwvg")
                nc.vector.tensor_scalar(cwv_g[:], cwv_raw[:],
                                        gate_col[:], None, ALU.mult)
                # denominator carry gets the +1e-6 folded in
                cw_g = ccp.tile([P, 8], f32, tag="cwg")
                nc.vector.tensor_scalar(cw_g[:], cw_raw[:],
                                        gate_col[:], 1e-6,
                                        ALU.mult, ALU.add)
                for c in range(8 * g, 8 * g + 8):
                    j = c - 8 * g
                    sw = pbp.tile([P, TL], bf16, tag="sw")
                    nc.vector.tensor_tensor_scan(
                        sw[:], wv_t[c][:], wv_t[c][:], cwv_g[:, j:j + 1],
                        ALU.add, ALU.bypass)
                    sw2 = pbp.tile([P, TL], bf16, tag="sw2")
                    nc.vector.tensor_tensor_scan(
                        sw2[:], w_t[c][:], w_t[c][:], cw_g[:, j:j + 1],
                        ALU.add, ALU.bypass)
                    qn = pbp.tile([P, TL], bf16, tag="qn")
                    nc.gpsimd.tensor_tensor(qn[:], q_sb[c][:], qscale[:],
                                            ALU.mult)
                    et = pbp.tile([P, TL], bf16, tag="et")
                    nc.scalar.activation(et[:], qn[:], AF.Exp, scale=-1.0)
                    # dd = (et + 1) * sw2   (sw2 already carries the +1e-6)
                    dd = pbp.tile([P, TL], bf16, tag="dd")
                    nc.vector.scalar_tensor_tensor(
                        dd[:], et[:], 1.0, sw2[:], ALU.add, ALU.mult)
                    lnd = pbp.tile([P, TL], bf16, tag="lnd")
                    nc.scalar.activation(lnd[:], dd[:], AF.Ln)
                    rr = pbp.tile([P, TL], bf16, tag="rr")
                    nc.scalar.activation(rr[:], lnd[:], AF.Exp, scale=-1.0)
                    yf = yfp.tile([P, TL], bf16, tag="yf")
                    nc.vector.tensor_tensor(yf[:], sw[:], rr[:], ALU.mult)
                    yf_t[c] = yf
                    # swiglu pass 1 (u half, m=0..7), c-interleaved
                    if c % 2 == 0:
                        w1 = _wgroup(nc, swp, wswiU, c, 2, tag="w1",
                                     name="w1")
                    for m in range(NE):
                        if c == 0:
                            sacc[m] = pss.tile([P, TL], f32, tag="sacc", name="sacc")
                        nc.tensor.matmul(sacc[m][:], w1[:, c % 2, m, :],
                                         yf[:],
                                         start=(c == 0), stop=(c == NC - 1))

            # drain u, then swiglu pass 2 (g half, m=8..15), c-outer
            u_sb = [None] * NE
            for m in range(NE):
                ut = up.tile([P, TL], bf16, tag="u")
                nc.scalar.copy(ut[:], sacc[m][:])
                u_sb[m] = ut
            sacc2 = [None] * NE
            for c in range(NC):
                if c % 2 == 0:
                    w2 = _wgroup(nc, swp, wswiG, c, 2, tag="w2", name="w2")
                for m in range(NE):
                    if c == 0:
                        sacc2[m] = pss.tile([P, TL], f32, tag="sacc", name="sacc2")
                    nc.tensor.matmul(sacc2[m][:], w2[:, c % 2, m, :],
                                     yf_t[c][:],
                                     start=(c == 0), stop=(c == NC - 1))
            # silu: m = u * g / (1 + exp(-g))
            m_t = [None] * NE
            for m in range(NE):
                eg = pbp.tile([P, TL], bf16, tag="eg")
                nc.scalar.activation(eg[:], sacc2[m][:], AF.Exp, scale=-1.0)
                lnd = pbp.tile([P, TL], bf16, tag="lnd")
                nc.scalar.activation(lnd[:], eg[:], AF.Ln,
                                     bias=const["oneb"][:])
                rr = pbp.tile([P, TL], bf16, tag="rr")
                nc.scalar.activation(rr[:], lnd[:], AF.Exp, scale=-1.0)
                pug = pbp.tile([P, TL], bf16, tag="pug")
                nc.vector.tensor_tensor(pug[:], u_sb[m][:], sacc2[m][:],
                                        ALU.mult)
                mt = mtp.tile([P, TL], bf16, tag="mt")
                nc.gpsimd.tensor_tensor(mt[:], pug[:], rr[:], ALU.mult)
                m_t[m] = mt

            # ---- out-proj + residual (SBUF resident) ----
            new_x = []
            with tc.tile_pool(name="a_w8b", bufs=2) as wpb:
                for mo in range(NE):
                    if mo % 4 == 0:
                        wo = _wgroup(nc, wpb, woutT, mo, 4, tag="wo",
                                     name="wo")
                    acc = pss.tile([P, TL], f32, tag="sacc", name="oacc")
                    for c in range(NE):
                        nc.tensor.matmul(acc[:], wo[:, mo % 4, c, :],
                                         m_t[c][:],
                                         start=(c == 0), stop=(c == NE - 1))
                    xo = xp.tile([P, TL], f32, tag="x", bufs=10)
                    nc.vector.tensor_tensor(xo[:], acc[:], x_tiles[mo][:],
                                            ALU.add)
                    new_x.append(xo)
                    if x3_bf is not None:
                        xob = pbp.tile([P, TL], bf16, tag="xob")
                        nc.scalar.copy(xob[:], xo[:])
                        nc.sync.dma_start(
                            x3_bf[mo * P:(mo + 1) * P, :], xob[:])
    return new_x


def _build_tea(tc, const, x_tiles, wqk4c, wv4c, wswiT4c, woutT4,
               agx_out_h, rs_in_q, rs_out_q, outT):
    nc = tc.nc
    ones_r = const["ones_r"]
    cc_t, ss_t, cm_t = const["cc"], const["ss"], const["cmask"]
    HL = 8

    with (
        tc.tile_pool(name="t_yt", bufs=2 * HL) as ytp,
        tc.tile_pool(name="t_sc", bufs=2) as scp,
        tc.tile_pool(name="t_ps", bufs=2, space="PSUM") as ps,
        tc.tile_pool(name="t_ps2", bufs=2, space="PSUM") as ps2,
        tc.tile_pool(name="t_xn", bufs=2 * NE) as xnp,
        tc.tile_pool(name="t_v", bufs=16) as vp,
    ):
        with tc.tile_pool(name="t_t", bufs=3) as sbp:
            # ---- rms(x3) (x3 arrives bf16 via the pair AllGather) ----
            xn = [[None] * NE for _ in range(2)]
            for tch in range(2):
                def _x3_ap(tch, e):
                    half, er = e // 4, e % 4
                    return agx_out_h[half].opt()[
                        tch * (E // 2) + er * P:tch * (E // 2) + (er + 1) * P, :]

                xt3s = []
                for e in range(NE):
                    xt3 = sbp.tile([P, TL], bf16, tag="xt3", bufs=NE)
                    nc.sync.dma_start(xt3[:], _x3_ap(tch, e))
                    xt3s.append(xt3)
                sumsq = ps2.tile([P, TL], f32, tag="sumsq")
                for e in range(NE):
                    xsq = sbp.tile([P, TL], bf16, tag="sq")
                    nc.gpsimd.tensor_tensor(xsq[:], xt3s[e][:], xt3s[e][:],
                                            ALU.mult)
                    nc.tensor.matmul(sumsq[:], const["ones_b"][:], xsq[:],
                                     start=(e == 0), stop=(e == NE - 1))
                xscale = _rsqrt(nc, scp, sumsq, 1.0 / E, const["epsb"][:],
                                "xscale", ln_bufs=2)
                for e in range(NE):
                    t = xnp.tile([P, TL], bf16, tag="xn")
                    nc.vector.tensor_tensor(t[:], xt3s[e][:], xscale[:],
                                            ALU.mult)
                    xn[tch][e] = t

            # ---- V (token-major) ----
            V = [[None] * 2 for _ in range(8)]
            with tc.tile_pool(name="t_vw", bufs=2) as vwp:
                for vb in range(2):
                    vw = vwp.tile([P, NE, TL], bf16, tag="vw")
                    nc.sync.dma_start(
                        vw[:],
                        wv4c.ap()[vb * P:(vb + 1) * P, :]
                        .rearrange("p (a n) -> p a n", n=TL))
                    for ttile in range(8):
                        tch, toff = ttile // 4, (ttile % 4) * P
                        acc = ps.tile([P, TL], f32, tag="mm")
                        for e in range(NE):
                            nc.tensor.matmul(
                                acc[:], xn[tch][e][:, toff:toff + P],
                                vw[:, e, :],
                                start=(e == 0), stop=(e == NE - 1))
                        vt = vp.tile([P, TL], bf16, tag="V")
                        nc.scalar.copy(vt[:], acc[:])
                        V[ttile][vb] = vt

        # ---- per-head rope/rms + attention ----
        yT = [[None] * 2 for _ in range(HL)]
        with (
            tc.tile_pool(name="t_qk", bufs=8) as qkp,
            tc.tile_pool(name="t_es", bufs=8) as esp,
            tc.tile_pool(name="t_w8", bufs=3) as wp,
            tc.tile_pool(name="t_at", bufs=2) as sba,
            tc.tile_pool(name="t_psa", bufs=2, space="PSUM") as psa,
            tc.tile_pool(name="t_psd", bufs=1, space="PSUM") as psd,
        ):
            sel4 = const["sel4"]

            def qk_phase(h):
                """QK matmuls + rope + rms-scale for head h; returns
                (qn_h, kn_h) f32r SBUF tiles."""
                qn_h = [None] * 2
                kn_h = [None] * 2
                sites = []
                coll = scp.tile([4, TL], f32, tag="coll", bufs=2,
                                name="coll")
                wqk = _wgroup(nc, wp, wqk4c, 2 * h, 2, tag="wqk", name="wqk")
                for wi, out_list in enumerate((qn_h, kn_h)):
                    for tch in range(2):
                        acc = ps.tile([P, TL], f32, tag="mm", name="acc")
                        for e in range(NE):
                            nc.tensor.matmul(acc[:], wqk[:, wi, e, :],
                                             xn[tch][e][:],
                                             start=(e == 0),
                                             stop=(e == NE - 1))
                        zsq = sba.tile([P, TL], f32r, tag="sq", name="zsq")
                        nc.scalar.activation(zsq[:], acc[:], AF.Square)
                        sq_ps = ps2.tile([1, TL], f32, tag="sumsq",
                                         name="sq_ps")
                        nc.tensor.matmul(sq_ps[:], ones_r[:, 0:1], zsq[:],
                                         start=True, stop=True)
                        r = 2 * wi + tch
                        srow = scp.tile([1, TL], f32, tag="srow", bufs=3,
                                        name="srow")
                        nc.scalar.copy(srow[:], sq_ps[:])
                        nc.sync.dma_start(coll[r:r + 1, :], srow[:])
                        tsl = slice(tch * TL, (tch + 1) * TL)
                        tmp1 = sba.tile([P, TL], f32, tag="tmp1",
                                        name="tmp1")
                        nc.vector.tensor_tensor(tmp1[:], acc[:],
                                                cc_t[:, tsl], ALU.mult)
                        cross = sba.tile([P, TL], f32, tag="cross",
                                         name="cross")
                        nc.vector.tensor_tensor(cross[:64, :], acc[64:, :],
                                                ss_t[:64, tsl], ALU.mult)
                        nc.vector.tensor_tensor(cross[64:, :], acc[:64, :],
                                                ss_t[64:, tsl], ALU.mult)
                        zrope = sba.tile([P, TL], f32, tag="zrope",
                                         bufs=6, name="zrope")
                        nc.gpsimd.tensor_tensor(zrope[:], tmp1[:], cross[:],
                                                ALU.add)
                        sites.append((r, zrope, out_list, tch))
                # one Ln + one Exp for all 4 sites of this head.
                lnc = scp.tile([4, TL], f32, tag="lnc", bufs=2, name="lnc")
                nc.scalar.activation(lnc[:], coll[:], AF.Ln,
                                     bias=const["epsbdh"][0:4, :])
                esc = scp.tile([4, TL], f32r, tag="esc", bufs=2, name="esc")
                nc.scalar.activation(esc[:], lnc[:], AF.Exp, scale=-0.5,
                                     bias=const["klnb"][:])
                for r, zrope, out_list, tch in sites:
                    sc_ps = ps.tile([P, TL], f32, tag="mm", name="sc_ps")
                    nc.tensor.matmul(sc_ps[:], sel4[:, r * P:(r + 1) * P],
                                     esc[:], start=True, stop=True)
                    zn = qkp.tile([P, TL], f32r, tag="zn", name="zn")
                    nc.vector.tensor_tensor(zn[:], zrope[:], sc_ps[:],
                                            ALU.mult)
                    out_list[tch] = zn
                return qn_h, kn_h

            def attn_phase(h, qn_h, kn_h):
                for qc in range(2):
                    denom = psd.tile([P, TL], f32, tag="denom")
                    ytil = psd.tile([P, TL], f32, tag="ytil")
                    nkt = 4 * (qc + 1)
                    for kt in range(nkt):
                        tch_k, koff = kt // 4, (kt % 4) * P
                        sT = psa.tile([P, TL], f32, tag="sT")
                        nc.tensor.matmul(sT[:],
                                         kn_h[tch_k][:, koff:koff + P],
                                         qn_h[qc][:], start=True, stop=True)
                        es = esp.tile([P, TL], bf16, tag="es")
                        j = kt - 4 * qc
                        if j >= 0:
                            sm = sba.tile([P, TL], f32, tag="sm")
                            nc.vector.tensor_tensor(
                                sm[:], sT[:], cm_t[:, j * TL:(j + 1) * TL],
                                ALU.add)
                            nc.scalar.activation(es[:], sm[:], AF.Exp)
                        else:
                            nc.scalar.activation(es[:], sT[:], AF.Exp)
                        nc.tensor.matmul(denom[:], const["ones_b"][:], es[:],
                                         start=(kt == 0),
                                         stop=(kt == nkt - 1))
                        nc.tensor.matmul(
                            ytil[:],
                            V[kt][h // 4][:, (h % 4) * P:(h % 4 + 1) * P],
                            es[:], start=(kt == 0), stop=(kt == nkt - 1))
                    lnr = sba.tile([P, TL], f32, tag="lnr")
                    nc.scalar.activation(lnr[:], denom[:], AF.Ln)
                    rr = sba.tile([P, TL], f32, tag="arr")
                    nc.scalar.activation(rr[:], lnr[:], AF.Exp, scale=-1.0)
                    yt = ytp.tile([P, TL], bf16, tag="yT")
                    nc.vector.tensor_tensor(yt[:], ytil[:], rr[:], ALU.mult)
                    yT[h][qc] = yt

            # software-pipeline: head h's norm-collection latency hides
            # under head h-1's attention matmuls
            pend = None
            for h in range(HL):
                qk = qk_phase(h)
                if pend is not None:
                    attn_phase(pend[0], *pend[1])
                pend = (h, qk)
            attn_phase(pend[0], *pend[1])

        # ---- partial swiglu, 4 ReduceScatter chunks pipelined ----
        # chunk j covers m-tiles {2j, 2j+1, 8+2j, 8+2j+1} (u-pair + g-pair);
        # wswiT4c is host-packed in exactly this consumption order.
        with (
            tc.tile_pool(name="t_w8s", bufs=2) as wps,
            tc.tile_pool(name="t_pug", bufs=4) as pugp,
        ):
            for j in range(4):
                wt = _wgroup(nc, wps, wswiT4c, 4 * j, 4, tag="ws",
                             name="ws")
                for s in range(4):
                    for tch in range(2):
                        acc = ps.tile([P, TL], f32, tag="mm")
                        for kk in range(HL):
                            nc.tensor.matmul(acc[:], wt[:, s, kk, :],
                                             yT[kk][tch][:],
                                             start=(kk == 0),
                                             stop=(kk == HL - 1))
                        pug = pugp.tile([P, TL], bf16, tag="pug")
                        nc.scalar.copy(pug[:], acc[:])
                        nc.sync.dma_start(
                            rs_in_q[j].opt()[tch * 4 * P + s * P:
                                             tch * 4 * P + (s + 1) * P, :],
                            pug[:])
                nc.gpsimd.collective_compute(
                    "ReduceScatter", ALU.add, replica_groups=PAIRS,
                    ins=[rs_in_q[j].opt()], outs=[rs_out_q[j].opt()])

        # ---- silu + out-proj (c-outer, overlaps RS chunks) + residual ----
        with (
            tc.tile_pool(name="t_mt", bufs=NE) as mtp,
            tc.tile_pool(name="t_w8o", bufs=1) as wpo,
            tc.tile_pool(name="t_t4", bufs=2) as sb4,
            tc.tile_pool(name="t_pso", bufs=4, space="PSUM") as pso,
        ):
            woc = wpo.tile([P, NE, NE, P], bf16, tag="woc", name="woc")
            nc.sync.dma_start(woc[:], woutT4.ap()
                              .rearrange("p (c b n) -> p c b n", c=NE, n=P))
            m_t = [None] * NE
            for j in range(4):
                for i in range(2):
                    c = 2 * j + i
                    ut = sb4.tile([P, TL], bf16, tag="u4")
                    nc.sync.dma_start(
                        ut[:], rs_out_q[j].opt()[i * P:(i + 1) * P, :])
                    gt = sb4.tile([P, TL], bf16, tag="g4")
                    nc.sync.dma_start(
                        gt[:],
                        rs_out_q[j].opt()[(2 + i) * P:(3 + i) * P, :])
                    eg = sb4.tile([P, TL], bf16, tag="eg4")
                    nc.scalar.activation(eg[:], gt[:], AF.Exp, scale=-1.0)
                    lnd = sb4.tile([P, TL], bf16, tag="lnd4")
                    nc.scalar.activation(lnd[:], eg[:], AF.Ln,
                                         bias=const["oneb"][:])
                    rr = sb4.tile([P, TL], bf16, tag="rr4")
                    nc.scalar.activation(rr[:], lnd[:], AF.Exp, scale=-1.0)
                    pug = sb4.tile([P, TL], bf16, tag="pug4")
                    nc.gpsimd.tensor_tensor(pug[:], ut[:], gt[:], ALU.mult)
                    mt = mtp.tile([P, TL], bf16, tag="mt4")
                    nc.vector.tensor_tensor(mt[:], pug[:], rr[:], ALU.mult)
                    m_t[c] = mt
            oacc = [None] * NE
            for half in range(2):
                for c in range(NE):
                    for mo in range(4 * half, 4 * half + 4):
                        if c == 0:
                            oacc[mo] = pso.tile([P, TL], f32, tag="oacc",
                                                name="oacc")
                        nc.tensor.matmul(oacc[mo][:], woc[:, c, mo, :],
                                         m_t[c][:],
                                         start=(c == 0), stop=(c == NE - 1))
                for mo in range(4 * half, 4 * half + 4):
                    xo = sb4.tile([P, TL], f32, tag="xo4")
                    nc.vector.tensor_tensor(xo[:], oacc[mo][:],
                                            x_tiles[mo][:], ALU.add)
                    nc.sync.dma_start(outT.ap()[mo * P:(mo + 1) * P, :],
                                      xo[:])


class _Bacc(bacc.Bacc):
    """Bacc with the combined ln+exp activation table given priority.

    The act-table insertion pass assigns each activation the first table
    in the list that contains its function; the default act_info order
    makes Exp resolve to `exp_and_others` and Ln to `natural_log`, so a
    kernel that alternates Exp/Ln (reciprocals, rsqrt) reloads the table
    on nearly every call (~1.3us each). Putting
    `natural_log_exp_and_others` first lets Exp/Ln/Square/Copy all share
    one resident table.
    """

    def insert_act_table_loads(self):
        import bass_rust as _bass_rust
        from concourse.hw_specs import get_activation_tables
        has_activation = any(
            isinstance(i, mybir.InstActivation)
            for b in self.main_func.blocks
            for i in b.instructions
        )
        if not has_activation:
            return
        steer = {AF.Exp, AF.Ln, AF.Square, AF.Copy}
        tables = [
            (nm, set(fns) if nm == 'natural_log_exp_and_others'
             else set(fns) - steer)
            for nm, fns in get_activation_tables(self.m.arch).items()
        ]
        _bass_rust.insert_act_table_loads(self, tables)


def build_program():
    nc = _Bacc("TRN2", target_bir_lowering=False, debug=False,
               num_devices=NCORES)

    din = {}

    def inp(name, shape, dt):
        din[name] = nc.dram_tensor(name, list(shape), dt,
                                   kind="ExternalInput")
        return din[name]

    inp("xT0", (E, TL), f32)
    for l in (1, 2, 3):
        inp(f"wqkvT{l}", (P, 48 * E), bf16)        # packed [p][m][e][n]
        inp(f"wswiU{l}", (P, NC * E), bf16)        # [p][c][m0..7][n] packed
        inp(f"wswiG{l}", (P, NC * E), bf16)        # [p][c][m8..15][n] packed
        inp(f"woutT{l}", (P, NE * E), bf16)        # packed [p][m][e][n]
    inp("wqk4c", (P, NC * E), bf16)                # packed [q_h0,k_h0,q_h1,..]
    inp("wv4c", (2 * P, NE * TL), bf16)            # [vb, p, e, n]
    inp("wswiT4c", (P, NC * E), bf16)              # packed in chunk order
    inp("woutT4", (P, NE * NE * P), bf16)          # [p][c][mo][n]
    inp("cc", (P, T), bf16)
    inp("ss", (P, T), bf16)
    inp("cmask", (P, 4 * TL), bf16)
    inp("gate", (P, 1), f32)
    inp("ones_r", (P, P), f32r)
    inp("ones_b", (P, P), bf16)
    inp("sel4", (4, 4 * P), f32r)
    inp("klnb", (4, 1), f32)
    outT = nc.dram_tensor("outT", [E, TL], f32, kind="ExternalOutput")

    with tile.TileContext(nc) as tc:
        with (
            tc.tile_pool(name="const", bufs=1) as constp,
            tc.tile_pool(name="xres", bufs=10) as xp,
            tc.tile_pool(name="dram", bufs=1, space="DRAM") as dram,
        ):
            const = {}
            epsb = constp.tile([P, 1], f32, tag="epsb")
            nc.any.memset(epsb[:], EPS)
            const["epsb"] = epsb
            epsbdh = constp.tile([P, 1], f32, tag="epsbdh")
            nc.any.memset(epsbdh[:], DH * EPS)
            const["epsbdh"] = epsbdh
            oneb = constp.tile([P, 1], f32, tag="oneb")
            nc.any.memset(oneb[:], 1.0)
            const["oneb"] = oneb
            for nm, dt in (("cc", bf16), ("ss", bf16), ("cmask", bf16),
                           ("gate", f32), ("ones_r", f32r),
                           ("ones_b", bf16), ("sel4", f32r), ("klnb", f32)):
                t = constp.tile(list(din[nm].shape), dt, tag=nm)
                nc.sync.dma_start(t[:], din[nm].ap())
                const[nm] = t

            # load residual stream into SBUF once
            x_tiles = []
            for e in range(NE):
                xt = xp.tile([P, TL], f32, tag="x", bufs=10)
                nc.sync.dma_start(xt[:], din["xT0"].ap()[e * P:(e + 1) * P, :])
                x_tiles.append(xt)

            agx_in = dram.tile([E, TL], bf16, tag="agx", name="agx")
            if True:
                for l in (1, 2, 3):
                    ag_ins = [dram.tile([1, 2048], f32, tag=f"agi{l}_{g}",
                                        name=f"agi{l}_{g}") for g in range(2)]
                    ag_outs = [dram.tile([2, 2048], f32, tag=f"ago{l}_{g}",
                                         name=f"ago{l}_{g}") for g in range(2)]
                    x_tiles = _build_aft_layer(
                        tc, const, x_tiles, xp,
                        din[f"wqkvT{l}"], din[f"wswiU{l}"], din[f"wswiG{l}"],
                        din[f"woutT{l}"],
                        ag_ins, ag_outs,
                        x3_bf=(agx_in.opt() if l == 3 else None))

            agx_out_h = [dram.tile([E, TL], bf16, tag=f"agxo{h}",
                                   name=f"agxo{h}") for h in range(2)]
            for half in range(2):
                nc.gpsimd.collective_compute(
                    "AllGather", ALU.bypass, replica_groups=PAIRS,
                    ins=[agx_in.opt()[half * (E // 2):(half + 1) * (E // 2), :]],
                    outs=[agx_out_h[half].opt()])
            rs_in_q = [dram.tile([NE * P, TL], bf16, tag=f"rsi{j}",
                                 name=f"rsi{j}") for j in range(4)]
            rs_out_q = [dram.tile([4 * P, TL], bf16, tag=f"rso{j}",
                                  name=f"rso{j}") for j in range(4)]
            _build_tea(tc, const, x_tiles, din["wqk4c"], din["wv4c"],
                       din["wswiT4c"], din["woutT4"], agx_out_h,
                       rs_in_q, rs_out_q, outT)

    nc.compile()
    return nc


# --------------------------------------------------------------------------
# host-side sharding
# --------------------------------------------------------------------------

def _host_inputs(inputs):
    f = np.float32
    cos = np.ascontiguousarray(np.asarray(inputs['cos'], f)[:, 0, :].T)
    sin = np.ascontiguousarray(np.asarray(inputs['sin'], f)[:, 0, :].T)
    cc = np.concatenate([cos, cos], 0)
    ss = np.concatenate([sin, -sin], 0)
    cm = np.zeros((4, P, TL), f)
    kk = np.arange(P)[:, None]
    qq = np.arange(TL)[None, :]
    for j in range(4):
        cm[j] = np.where(P * j + kk <= qq, 0.0, -1e30)
    cmask = np.ascontiguousarray(cm.transpose(1, 0, 2).reshape(P, 4 * TL))
    ones_r = np.ones((P, P), f)
    ones_b = np.ones((P, P), BF)

    def tl(wT):
        # (K, M) -> tile layout (M, K): row-block m = [p, e, n] contiguous
        K, M = wT.shape
        return np.ascontiguousarray(
            wT.reshape(K // P, P, M // P, P).transpose(2, 1, 0, 3)
            .reshape(M, K))

    def pk(wT, perm=None):
        # (K, M) -> (P, (M/P)*K) packed: row p holds [m][e][n] contiguous,
        # so a G-tile DMA is one G*K-elem chunk per partition.
        K, M = wT.shape
        t = tl(wT).reshape(M // P, P, K).transpose(1, 0, 2)  # (P, m, K)
        if perm is not None:
            t = t[:, perm, :]
        return np.ascontiguousarray(t.reshape(P, (M // P) * K))

    sel4 = np.zeros((4, 4 * P), f)
    for i in range(4):
        sel4[i, i * P:(i + 1) * P] = 1.0
    klnb = np.array([[0.0], [0.0], [0.5 * np.log(DH)], [0.5 * np.log(DH)]], f)
    shared = {'cc': cc.astype(BF), 'ss': ss.astype(BF),
              'cmask': cmask.astype(BF), 'ones_r': ones_r,
              'ones_b': ones_b, 'sel4': sel4, 'klnb': klnb}
    for l in (1, 2, 3):
        shared[f'wqkvT{l}'] = pk(np.asarray(inputs[f'w_qkv{l}'], f).T).astype(BF)
        wswiT = np.asarray(inputs[f'w_swiglu{l}'], f).T   # (QKV, 2E)
        # [p][c][m][n] packing of each half: swiglu pass-1/2 c-tile loads
        for nm, half in (('wswiU', wswiT[:, :E]), ('wswiG', wswiT[:, E:])):
            shared[f'{nm}{l}'] = np.ascontiguousarray(
                half.reshape(NC, P, NE, P).transpose(1, 0, 2, 3)
                .reshape(P, NC * E)).astype(BF)
        shared[f'woutT{l}'] = pk(np.asarray(inputs[f'w_out{l}'], f).T).astype(BF)
    wout4T = np.asarray(inputs['w_out4'], f).T             # (E, E)
    # [p][c][mo][n] c-major packing for the c-outer TEA out-projection
    shared['woutT4'] = np.ascontiguousarray(
        wout4T.reshape(NE, P, NE, P).transpose(1, 0, 2, 3)
        .reshape(P, NE * E)).astype(BF)

    wq4 = np.asarray(inputs['w_qkv4'], f).T       # (E, 6144): per-head blocks
    wswi4 = np.asarray(inputs['w_swiglu4'], f).T  # (QKV, 2E)
    by_par = {}
    for par in range(2):
        hs = par * 8
        qk_cols = []
        for h in range(hs, hs + 8):     # interleaved [q_h, k_h] pairs
            for part in range(2):
                qk_cols.append(wq4[:, h * 3 * DH + part * DH:
                                   h * 3 * DH + (part + 1) * DH])
        v_cols = [wq4[:, h * 3 * DH + 2 * DH: h * 3 * DH + 3 * DH]
                  for h in range(hs, hs + 8)]
        kv = np.concatenate(v_cols, 1)             # (E, 1024)
        # wv4c layout [vb, p, e, n]: element = kv[128e + p, vb*512 + n]
        wv4c = np.ascontiguousarray(
            kv.reshape(NE, P, 2, TL).transpose(2, 1, 0, 3)
            .reshape(2 * P, NE * TL))
        # chunk-order permutation for the 4-way ReduceScatter pipeline
        swi_perm = [m for j in range(4)
                    for m in (2 * j, 2 * j + 1, 8 + 2 * j, 9 + 2 * j)]
        by_par[par] = {
            'wqk4c': pk(np.concatenate(qk_cols, 1)).astype(BF),
            'wv4c': wv4c.astype(BF),
            'wswiT4c': pk(np.ascontiguousarray(
                wswi4[hs * DH:(hs + 8) * DH, :]), perm=swi_perm).astype(BF),
            'gate': np.full((P, 1), float(par), f),
        }

    x = np.asarray(inputs['x'], f)
    in_maps = []
    for c in range(NCORES):
        b, par = c // 2, c % 2
        m = dict(shared)
        m.update(by_par[par])
        m['xT0'] = np.ascontiguousarray(x[b, par * TL:(par + 1) * TL, :].T)
        in_maps.append(m)
    return in_maps


_cached = {}


def kernel(**inputs):
    if 'nc' not in _cached:
        _cached['nc'] = build_program()
    nc = _cached['nc']
    in_maps = _host_inputs(inputs)
    trace = bool(int(os.environ.get('BASS_KERNEL_TRACE', '0')))
    res = run_bass_kernel_spmd(nc, in_maps, core_ids=list(range(NCORES)),
                               trace=trace)
    _cached['last_results'] = res
    out = np.zeros((4, T, E), np.float32)
    for c in range(NCORES):
        b, par = c // 2, c % 2
        out[b, par * TL:(par + 1) * TL, :] = res.results[c]['outT'].T
    return out
